# revision 4
# baseline (speedup 1.0000x reference)
"""BitNet FFN (bitlinear158 -> gelu -> bitlinear158) Trainium2 kernel.

Sharding: data-parallel over tokens across 8 cores (1024 tokens/core).
Layout: tokens on the free axis everywhere; weights stationary in the PE.

v3 design:
  - No weight AllGathers: each core streams the FULL weights from its own
    DRAM copy in strip-major contiguous layout (8KB DMA rows) and
    quantizes strips locally, pipelined under the matmuls.
  - Only collective: one tiny AllReduce per layer for the weight-scale
    partial sum (mean|W| must be global-exact); trigger issued early,
    result readback deferred so no engine queue blocks on it.
  - x strips stay resident in SBUF between the stats pass and the quant
    pass (no second HBM read of x).
  - All round-to-int ops (C_ROUND add/sub) run on GPSIMD; clips/scales
    on vector, squares/gelu/weight-scale on scalar: every engine stays
    below the PE's per-strip budget.
  - Layer-2 h quantization interleaved with m=0's weight chunks so the
    PE k-accumulation of the first output group consumes hqT strips as
    they are produced.

Math notes (exactness, same recipe as baseline):
  - activation quant ints = round(x * 127 / max|x|)  (the rms-norm cancels)
  - weight quant ternary = clip(round(w / clip(mean|w|,1e-5)), -1, 1)
  - both exactly representable in bf16; PSUM accumulates integer products
    (<= 2^21) exactly in fp32, so the matmuls are exact.
  - per-token output scale alpha = clip(max|x|*sqrt(d)/||x||, 1e-5)
      * clip(mean|w|,1e-5) / 127 applied on PSUM before gelu.
  - round-to-nearest-even via fp32 (t + 1.5*2**23) - 1.5*2**23, matching
    jnp.round; clip(round(t),-1,1) == round(clamp(t, +-1.4999999)).
"""

import sys

for _p in ("/opt/trn_rl_repo", "/opt/trn_rl_repo/concourse"):
    if _p not in sys.path:
        sys.path.insert(0, _p)

import numpy as np

import concourse.bass as bass
import concourse.bacc as bacc
import concourse.mybir as mybir
import concourse.tile as tile
from concourse import library_config
from concourse.bass import ts
from concourse.masks import make_identity

F32 = mybir.dt.float32
BF16 = mybir.dt.bfloat16
AX = mybir.AxisListType.X
OP = mybir.AluOpType
AF = mybir.ActivationFunctionType

C_ROUND = 12582912.0  # 1.5 * 2**23 : fp32 RNE rounding constant
W_CLIP = 1.4999999    # round(clamp(t, +-W_CLIP)) == clip(round(t), -1, 1)
N_CORES = 8


def build_bitnet(D, I, T, n_cores=N_CORES):
    """Per-core SPMD Bass program.

    Per-core I/O: xT [D,T] f32 (token shard, transposed); w1p
    [I/128,128,(D/128)*128] f32 and w2p [D/256,128,(I/128)*256] f32
    (full weights, strip-major); w1sh/w2sh (this core's 1/n slice of
    each, for the mean|w| partial) -> outT [D,T] f32.
    """
    KD = D // 128    # d tiles (layer-1 contraction; layer-2 output side)
    KI = I // 128    # inner tiles
    TH = T // 2      # matmul moving free dim
    TJ = T // 128    # token tiles for stats transposes
    NM = D // 256    # layer-2 output strips (256 out rows each)
    NSH1 = KI // n_cores   # w1 strips per core for the scale partial
    CW = 2048        # w2 quant chunk width (f32 elems per partition)
    NCH = (KI * 256) // CW   # w2 chunks per m strip (= 8)
    KPC = CW // 256          # k-steps per w2 chunk (= 8)
    inv_cnt = 1.0 / float(D * I)
    sqrt_d = float(np.sqrt(np.float64(D)))
    sqrt_i = float(np.sqrt(np.float64(I)))

    nc = bacc.Bacc("TRN2", num_devices=n_cores)

    xT = nc.dram_tensor("xT", [D, T], F32, kind="ExternalInput")
    w1p = nc.dram_tensor("w1p", [KI, 128, KD * 128], F32,
                         kind="ExternalInput")
    w2p = nc.dram_tensor("w2p", [NM, 128, KI * 256], F32,
                         kind="ExternalInput")
    w1sh = nc.dram_tensor("w1sh", [NSH1, 128, KD * 128], F32,
                          kind="ExternalInput")
    w2sh = nc.dram_tensor("w2sh", [128, KI * 256], F32,
                          kind="ExternalInput")
    outT = nc.dram_tensor("outT", [D, T], F32, kind="ExternalOutput")

    h_dram = nc.dram_tensor("h_scratch", [KI, 128, T], F32, kind="Internal")
    ar1_in = nc.dram_tensor("ar1_in", [8], F32, kind="Internal")
    ar1_out = nc.dram_tensor("ar1_out", [8], F32, kind="Internal",
                             addr_space="Shared")
    ar2_in = nc.dram_tensor("ar2_in", [8], F32, kind="Internal")
    ar2_out = nc.dram_tensor("ar2_out", [8], F32, kind="Internal",
                             addr_space="Shared")
    stat_dram = nc.dram_tensor("stat_dram", [6, T], F32, kind="Internal")
    srow_v = stat_dram.ap()                                       # [6, T]
    stok_v = stat_dram.ap().rearrange("r (j p) -> r p j", p=128)  # [6,128,TJ]

    xT_t = xT.ap().rearrange("(k p) t -> k p t", p=128)           # [KD,128,T]
    w1p_v = w1p.ap()
    w2p_v = w2p.ap()
    w1sh_v = w1sh.ap()
    w2sh_v = w2sh.ap()
    h_w = h_dram.ap()
    out_w = outT.ap().rearrange("(k p) t -> k p t", p=128)

    with tile.TileContext(nc) as tc:
        with (
            tc.tile_pool(name="glob", bufs=1) as glob,
            tc.tile_pool(name="psum", bufs=8, space="PSUM") as psum,
        ):
            # --- persistent constants & broadcast tiles ---
            ident = glob.tile([128, 128], F32)
            make_identity(nc, ident)
            wsc1 = glob.tile([128, 2], F32, tag="wsc1")  # cols: s1, mclip1
            wsc2 = glob.tile([128, 2], F32, tag="wsc2")
            qs1_b = glob.tile([128, T], F32, tag="qsb")
            al1_b = glob.tile([128, T], F32, tag="alb")

            with tc.tile_pool(name="stats", bufs=1) as stats:

                def part_reduce(acc, res, op):
                    # reduce [128, T] over partitions -> res [128,TJ] tok-part
                    for j in range(TJ):
                        trp = psum.tile([128, 128], F32, tag="b", name="trp")
                        nc.tensor.transpose(trp[:, :], acc[:, ts(j, 128)],
                                            ident[:, :])
                        nc.vector.tensor_reduce(
                            out=res[:, j:j + 1], in_=trp[:, :], axis=AX, op=op)

                def qs_part(Mx, r0, qs_b):
                    # qs_b = broadcast(127 / max|x| per token) [128, T]
                    qs = stats.tile([128, TJ], F32, tag="qs", name="qs")
                    nc.vector.tensor_scalar(qs, Mx, 1e-30, None, OP.max)
                    nc.vector.reciprocal(qs, qs)
                    nc.vector.tensor_scalar(qs, qs, 127.0, None, OP.mult)
                    nc.sync.dma_start(out=stok_v[r0 + 1], in_=qs[:, :])
                    qrow = stats.tile([1, T], F32, tag="qrow", name="qrow")
                    nc.sync.dma_start(out=qrow[:, :],
                                      in_=srow_v[r0 + 1:r0 + 2, :])
                    nc.gpsimd.partition_broadcast(qs_b[:, :], qrow[:, :])

                def al_part(Mx, ssq, wsc, sqrt_dim, r0, al_b):
                    # al_b = broadcast(per-token dequant scale) [128, T]
                    nrm = stats.tile([128, TJ], F32, tag="nrm", name="nrm")
                    nc.vector.tensor_scalar(nrm, ssq, 1e-38, None, OP.max)
                    nc.scalar.activation(nrm, nrm, AF.Sqrt)
                    nc.vector.tensor_scalar(nrm, nrm, 1e-12, None, OP.max)
                    inv_n = stats.tile([128, TJ], F32, tag="invn",
                                       name="inv_n")
                    nc.vector.reciprocal(inv_n, nrm)
                    al = stats.tile([128, TJ], F32, tag="al", name="al")
                    nc.vector.tensor_tensor(al, Mx, inv_n, OP.mult)
                    nc.vector.tensor_scalar(al, al, sqrt_dim, 1e-5,
                                            OP.mult, OP.max)
                    nc.vector.tensor_scalar(al, al, wsc[:, 1:2], 1.0 / 127.0,
                                            OP.mult, OP.mult)
                    nc.sync.dma_start(out=stok_v[r0 + 2], in_=al[:, :])
                    arow = stats.tile([1, T], F32, tag="arow", name="arow")
                    nc.sync.dma_start(out=arow[:, :],
                                      in_=srow_v[r0 + 2:r0 + 3, :])
                    nc.gpsimd.partition_broadcast(al_b[:, :], arow[:, :])

                def scale_trigger(wps, ar_in, ar_out):
                    # column partial sums -> scalar -> AllReduce trigger
                    wpad = stats.tile([128, 128], F32, tag="wpad",
                                      name="wpad")
                    nc.vector.memset(wpad, 0.0)
                    nc.vector.reduce_sum(wpad[:, 0:1], wps, axis=AX)
                    trw = psum.tile([128, 128], F32, tag="b", name="trw")
                    nc.tensor.transpose(trw[:, :], wpad[:, :], ident[:, :])
                    wred = stats.tile([8, 1], F32, tag="wred", name="wred")
                    nc.vector.memset(wred, 0.0)
                    nc.vector.reduce_sum(wred[0:1, :], trw[0:1, :], axis=AX)
                    nc.sync.dma_start(out=ar_in.ap()[0:8], in_=wred[:, :])
                    nc.gpsimd.collective_compute(
                        "AllReduce", OP.add,
                        replica_groups=[list(range(n_cores))],
                        ins=[ar_in.ap().opt()], outs=[ar_out.ap().opt()])

                def scale_readback(ar_out, wsc):
                    wrow = stats.tile([1, 1], F32, tag="wrow", name="wrow")
                    nc.sync.dma_start(out=wrow[:, :], in_=ar_out.ap()[0:1])
                    mrow = stats.tile([1, 2], F32, tag="mrow", name="mrow")
                    nc.vector.tensor_scalar(mrow[:, 1:2], wrow, inv_cnt,
                                            1e-5, OP.mult, OP.max)
                    nc.vector.reciprocal(mrow[:, 0:1], mrow[:, 1:2])
                    nc.gpsimd.partition_broadcast(wsc[:, :], mrow[:, :])

                with tc.tile_pool(name="bc", bufs=2) as bc:
                    # -- x strip DMAs first (kept resident in SBUF) --
                    xk_tiles = {}
                    for k in range(KD):
                        xk = bc.tile([128, T], F32, tag="xk", bufs=KD,
                                     name="xk")
                        nc.sync.dma_start(out=xk[:, :], in_=xT_t[k])
                        xk_tiles[k] = xk

                    # -- w1 scale-partial shard reads (share w1f ring) --
                    wps1 = stats.tile([128, NSH1], F32, tag="wps1",
                                      name="wps1")
                    for a in range(NSH1):
                        wtmp = bc.tile([128, KD * 128], F32, tag="w1f",
                                       bufs=4, name="wtmp")
                        nc.sync.dma_start(out=wtmp[:, :], in_=w1sh_v[a])
                        nc.vector.tensor_reduce(
                            out=wps1[:, a:a + 1], in_=wtmp[:, :], axis=AX,
                            op=OP.add, apply_absolute_value=True)

                    # -- prefetch first layer-1 weight strips --
                    w1f_tiles = {}

                    def load_w1f(i):
                        t = bc.tile([128, KD * 128], F32, tag="w1f", bufs=4,
                                    name="w1f")
                        nc.sync.dma_start(out=t[:, :], in_=w1p_v[i])
                        w1f_tiles[i] = t

                    for i in range(4):
                        load_w1f(i)

                    # ====== Stage B: x stats ======
                    am1p = stats.tile([128, T], F32, tag="amp", name="am1p")
                    am1n = stats.tile([128, T], F32, tag="amn", name="am1n")
                    sq1 = stats.tile([128, T], F32, tag="sq", name="sq1")
                    for k in range(KD):
                        xk = xk_tiles[k]
                        if k == 0:
                            nc.vector.tensor_copy(am1p, xk)
                            nc.vector.tensor_copy(am1n, xk)
                        else:
                            nc.vector.tensor_tensor(am1p, xk, am1p, OP.max)
                            nc.vector.tensor_tensor(am1n, xk, am1n, OP.min)
                        xsq = bc.tile([128, T], F32, tag="xsq", name="xsq")
                        nc.scalar.activation(xsq, xk, AF.Square)
                        if k == 0:
                            nc.gpsimd.tensor_copy(sq1, xsq)
                        else:
                            nc.gpsimd.tensor_tensor(sq1, xsq, sq1, OP.add)
                    nc.vector.scalar_tensor_tensor(
                        am1n, am1n, -1.0, am1p, OP.mult, OP.max)
                    Mx1 = stats.tile([128, TJ], F32, tag="Mx", name="Mx1")
                    part_reduce(am1n, Mx1, OP.max)
                    Sq1 = stats.tile([128, TJ], F32, tag="Sq", name="Sq1")
                    part_reduce(sq1, Sq1, OP.add)
                    qs_part(Mx1, 0, qs1_b)
                    # AllReduce1: partial was reduced above; trigger + read
                    scale_trigger(wps1, ar1_in, ar1_out)
                    scale_readback(ar1_out, wsc1)
                    al_part(Mx1, Sq1, wsc1, sqrt_d, 0, al1_b)

                    # ====== Stage B2: x quant (in SBUF, no re-read) ======
                    xqT = bc.tile([128, KD, T], BF16, tag="xqT", bufs=1,
                                  name="xqT")
                    for k in range(KD):
                        xk = xk_tiles.pop(k)
                        nc.vector.tensor_tensor(xk, xk, qs1_b, OP.mult)
                        nc.gpsimd.tensor_scalar(xqT[:, k, :], xk, C_ROUND,
                                                C_ROUND, OP.add, OP.subtract)

                    # ====== Stage C: layer 1 (+ w2 scale interleaved) ======
                    am2p = stats.tile([128, T], F32, tag="amp", name="am2p")
                    am2n = stats.tile([128, T], F32, tag="amn", name="am2n")
                    sq2 = stats.tile([128, T], F32, tag="sq", name="sq2")
                    wps2 = stats.tile([128, NCH], F32, tag="wps2",
                                      name="wps2")
                    for i in range(KI):
                        w1f = w1f_tiles.pop(i)
                        w1ff = w1f[:, :]
                        nc.scalar.activation(w1ff, w1ff, AF.Copy,
                                             scale=wsc1[:, 0:1])
                        nc.vector.tensor_scalar(w1ff, w1ff, W_CLIP, -W_CLIP,
                                                OP.min, OP.max)
                        w1q = bc.tile([128, KD, 128], BF16, tag="w1q",
                                      bufs=4, name="w1q")
                        nc.gpsimd.tensor_scalar(
                            w1q.rearrange("p k j -> p (k j)"), w1ff, C_ROUND,
                            C_ROUND, OP.add, OP.subtract)
                        if i + 4 < KI:
                            load_w1f(i + 4)
                        hpsA = psum.tile([128, TH], F32, tag="b",
                                         name="hpsA")
                        hpsB = psum.tile([128, TH], F32, tag="b",
                                         name="hpsB")
                        for k in range(KD):
                            nc.tensor.matmul(hpsA[:, :], w1q[:, k, :],
                                             xqT[:, k, 0:TH],
                                             start=(k == 0),
                                             stop=(k == KD - 1))
                        for k in range(KD):
                            nc.tensor.matmul(hpsB[:, :], w1q[:, k, :],
                                             xqT[:, k, TH:T],
                                             start=(k == 0),
                                             stop=(k == KD - 1))
                        nc.vector.tensor_tensor(hpsA, hpsA, al1_b[:, 0:TH],
                                                OP.mult)
                        nc.vector.tensor_tensor(hpsB, hpsB, al1_b[:, TH:T],
                                                OP.mult)
                        h_sb = bc.tile([128, T], F32, tag="h", bufs=3,
                                       name="h_sb")
                        nc.scalar.activation(h_sb[:, 0:TH], hpsA, AF.Gelu)
                        nc.scalar.activation(h_sb[:, TH:T], hpsB, AF.Gelu)
                        nc.sync.dma_start(out=h_w[i], in_=h_sb[:, :])
                        if i == 0:
                            nc.vector.tensor_copy(am2p, h_sb)
                            nc.vector.tensor_copy(am2n, h_sb)
                        else:
                            nc.vector.tensor_tensor(am2p, h_sb, am2p, OP.max)
                            nc.vector.tensor_tensor(am2n, h_sb, am2n, OP.min)
                        hsq = bc.tile([128, T], F32, tag="hsq", name="hsq")
                        nc.scalar.activation(hsq, h_sb, AF.Square)
                        if i == 0:
                            nc.gpsimd.tensor_copy(sq2, hsq)
                        else:
                            nc.gpsimd.tensor_tensor(sq2, hsq, sq2, OP.add)
                        # interleaved w2 scale partial (this core's strip)
                        if i < NCH:
                            wtmp2 = bc.tile([128, CW], F32, tag="w1f",
                                            bufs=4, name="wtmp2")
                            nc.sync.dma_start(
                                out=wtmp2[:, :],
                                in_=w2sh_v[:, CW * i:CW * (i + 1)])
                            nc.vector.tensor_reduce(
                                out=wps2[:, i:i + 1], in_=wtmp2[:, :],
                                axis=AX, op=OP.add,
                                apply_absolute_value=True)
                        elif i == NCH:
                            scale_trigger(wps2, ar2_in, ar2_out)
                        elif i == 56:
                            scale_readback(ar2_out, wsc2)

                    # ---- mid stats finalize ----
                    qs2_b = glob.tile([128, T], F32, tag="qsb",
                                      name="qs2_b")
                    al2_b = glob.tile([128, T], F32, tag="alb",
                                      name="al2_b")
                    nc.vector.scalar_tensor_tensor(
                        am2n, am2n, -1.0, am2p, OP.mult, OP.max)
                    Mx2 = stats.tile([128, TJ], F32, tag="Mx", name="Mx2")
                    part_reduce(am2n, Mx2, OP.max)
                    Sq2 = stats.tile([128, TJ], F32, tag="Sq", name="Sq2")
                    part_reduce(sq2, Sq2, OP.add)
                    qs_part(Mx2, 3, qs2_b)
                    al_part(Mx2, Sq2, wsc2, sqrt_i, 3, al2_b)

            # ============ Stage D: quantize h, layer 2 ============
            with tc.tile_pool(name="l2", bufs=2) as l2:
                w2qc_tiles = {}

                def issue_w2_chunk(m, q):
                    w2f = l2.tile([128, CW], F32, tag="w2f", bufs=2,
                                  name="w2f")
                    nc.sync.dma_start(
                        out=w2f[:, :],
                        in_=w2p_v[m][:, CW * q:CW * (q + 1)])
                    nc.scalar.activation(w2f[:, :], w2f[:, :], AF.Copy,
                                         scale=wsc2[:, 0:1])
                    nc.vector.tensor_scalar(w2f, w2f, W_CLIP, -W_CLIP,
                                            OP.min, OP.max)
                    t = l2.tile([128, KPC, 256], BF16, tag="w2q", bufs=8,
                                name="w2q")
                    nc.vector.tensor_scalar(
                        t.rearrange("p a c -> p (a c)"), w2f, C_ROUND,
                        C_ROUND, OP.add, OP.subtract)
                    w2qc_tiles[(m, q)] = t

                hqT = l2.tile([128, KI, T], BF16, tag="hqT", bufs=1,
                              name="hqT")
                for k2 in range(KI):
                    hk = l2.tile([128, T], F32, tag="hrd", name="hk")
                    nc.sync.dma_start(out=hk[:, :], in_=h_w[k2])
                    nc.vector.tensor_tensor(hk, hk, qs2_b, OP.mult)
                    nc.gpsimd.tensor_scalar(hqT[:, k2, :], hk, C_ROUND,
                                            C_ROUND, OP.add, OP.subtract)
                    if k2 % KPC == KPC - 1:
                        issue_w2_chunk(0, k2 // KPC)

                for m in range(NM):
                    if m > 0:
                        for q in range(NCH):
                            issue_w2_chunk(m, q)
                    pb = [psum.tile([128, TH], F32, tag="b",
                                    name=f"pb{qq}") for qq in range(4)]
                    for kk in range(KI):
                        ch = w2qc_tiles.pop((m, kk // KPC)) \
                            if kk % KPC == KPC - 1 else \
                            w2qc_tiles[(m, kk // KPC)]
                        r = kk % KPC
                        first = (kk == 0)
                        last = (kk == KI - 1)
                        nc.tensor.matmul(pb[0][:, :], ch[:, r, 0:128],
                                         hqT[:, kk, 0:TH],
                                         start=first, stop=last)
                        nc.tensor.matmul(pb[1][:, :], ch[:, r, 0:128],
                                         hqT[:, kk, TH:T],
                                         start=first, stop=last)
                        nc.tensor.matmul(pb[2][:, :], ch[:, r, 128:256],
                                         hqT[:, kk, 0:TH],
                                         start=first, stop=last)
                        nc.tensor.matmul(pb[3][:, :], ch[:, r, 128:256],
                                         hqT[:, kk, TH:T],
                                         start=first, stop=last)
                    for qq in range(4):
                        jcol = qq >> 1
                        half = qq & 1
                        ob = l2.tile([128, TH], F32, tag="ob", bufs=3,
                                     name="ob")
                        nc.vector.tensor_tensor(
                            ob, pb[qq], al2_b[:, ts(half, TH)], OP.mult)
                        nc.sync.dma_start(
                            out=out_w[2 * m + jcol][:, ts(half, TH)],
                            in_=ob[:, :])

    nc.compile()  # Bacc passes: EVSEM multi-wait lowering, library loads,
    return nc     # extended-ISA codegen, nop fusion, register alloc


_NC_CACHE = {}


def _get_nc(D, I, T, n_cores):
    key = (D, I, T, n_cores)
    if key not in _NC_CACHE:
        _NC_CACHE[key] = build_bitnet(D, I, T, n_cores)
    return _NC_CACHE[key]


def make_in_maps(x, w1, w2, n_cores=N_CORES):
    """Host-side sharding/layout only (transpose + slicing, no arithmetic)."""
    xf = np.ascontiguousarray(np.asarray(x, dtype=np.float32)).reshape(
        -1, x.shape[-1])
    D = xf.shape[1]
    I = w1.shape[0]
    T = xf.shape[0] // n_cores
    KD = D // 128
    KI = I // 128
    NM = D // 256
    w1np = np.asarray(w1, dtype=np.float32)
    w2np = np.asarray(w2, dtype=np.float32)
    # strip-major stationary layouts (pure element permutations)
    # w1p[s, p, k*128+j] = w1[128s+j, 128k+p]
    w1p = np.ascontiguousarray(
        w1np.reshape(KI, 128, KD, 128).transpose(0, 3, 2, 1)).reshape(
        KI, 128, KD * 128)
    # w2p[m, p, k*256+c] = w2[256m+c, 128k+p]
    w2p = np.ascontiguousarray(
        w2np.reshape(NM, 256, KI, 128).transpose(0, 3, 2, 1)).reshape(
        NM, 128, KI * 256)
    in_maps = []
    for c in range(n_cores):
        xTc = np.ascontiguousarray(xf[c * T:(c + 1) * T].T)  # [D, T]
        in_maps.append({
            "xT": xTc,
            "w1p": w1p,
            "w2p": w2p,
            "w1sh": w1p[c * (KI // n_cores):(c + 1) * (KI // n_cores)],
            "w2sh": w2p[c],
        })
    return in_maps, (D, I, T)


def run_spmd(x, w1, w2, trace=False, **kwargs):
    from concourse.bass_utils import run_bass_kernel_spmd

    B, S, D = x.shape
    in_maps, (D, I, T) = make_in_maps(x, w1, w2, N_CORES)
    nc = _get_nc(D, I, T, N_CORES)
    res = run_bass_kernel_spmd(nc, in_maps, core_ids=list(range(N_CORES)),
                               trace=trace, **kwargs)
    outs = [res.results[c]["outT"].T for c in range(N_CORES)]  # each [T, D]
    out = np.concatenate(outs, axis=0).reshape(B, S, D)
    return np.ascontiguousarray(out, dtype=np.float32), res

def kernel(x, w1, w2):
    out, _ = run_spmd(x, w1, w2, trace=False)
    return out


# revision 5
# speedup vs baseline: 2.8299x; 2.8299x over previous
"""BitNet FFN (bitlinear158 -> gelu -> bitlinear158) Trainium2 kernel.

Sharding: data-parallel over tokens across 8 cores (1024 tokens/core).
Layout: tokens on the free axis everywhere; weights stationary in the PE.

v3 design:
  - No weight AllGathers: each core streams the FULL weights from its own
    DRAM copy in strip-major contiguous layout (8KB DMA rows) and
    quantizes strips locally, pipelined under the matmuls.
  - Only collective: one tiny AllReduce per layer for the weight-scale
    partial sum (mean|W| must be global-exact); trigger issued early,
    result readback deferred so no engine queue blocks on it.
  - x strips stay resident in SBUF between the stats pass and the quant
    pass (no second HBM read of x).
  - All round-to-int ops (C_ROUND add/sub) run on GPSIMD; clips/scales
    on vector, squares/gelu/weight-scale on scalar: every engine stays
    below the PE's per-strip budget.
  - Layer-2 h quantization interleaved with m=0's weight chunks so the
    PE k-accumulation of the first output group consumes hqT strips as
    they are produced.

Math notes (exactness, same recipe as baseline):
  - activation quant ints = round(x * 127 / max|x|)  (the rms-norm cancels)
  - weight quant ternary = clip(round(w / clip(mean|w|,1e-5)), -1, 1)
  - both exactly representable in bf16; PSUM accumulates integer products
    (<= 2^21) exactly in fp32, so the matmuls are exact.
  - per-token output scale alpha = clip(max|x|*sqrt(d)/||x||, 1e-5)
      * clip(mean|w|,1e-5) / 127 applied on PSUM before gelu.
  - round-to-nearest-even via fp32 (t + 1.5*2**23) - 1.5*2**23, matching
    jnp.round; clip(round(t),-1,1) == round(clamp(t, +-1.4999999)).
"""

import sys

for _p in ("/opt/trn_rl_repo", "/opt/trn_rl_repo/concourse"):
    if _p not in sys.path:
        sys.path.insert(0, _p)

import numpy as np

import concourse.bass as bass
import concourse.bacc as bacc
import concourse.mybir as mybir
import concourse.tile as tile
from concourse import library_config
from concourse.bass import ts
from concourse.masks import make_identity

F32 = mybir.dt.float32
BF16 = mybir.dt.bfloat16
AX = mybir.AxisListType.X
OP = mybir.AluOpType
AF = mybir.ActivationFunctionType

C_ROUND = 12582912.0  # 1.5 * 2**23 : fp32 RNE rounding constant
W_CLIP = 1.4999999    # round(clamp(t, +-W_CLIP)) == clip(round(t), -1, 1)
N_CORES = 8


def build_bitnet(D, I, T, n_cores=N_CORES):
    """Per-core SPMD Bass program.

    Per-core I/O: xT [D,T] f32 (token shard, transposed); w1p
    [I/128,128,(D/128)*128] f32 and w2p [D/256,128,(I/128)*256] f32
    (full weights, strip-major); w1sh/w2sh (this core's 1/n slice of
    each, for the mean|w| partial) -> outT [D,T] f32.
    """
    KD = D // 128    # d tiles (layer-1 contraction; layer-2 output side)
    KI = I // 128    # inner tiles
    TH = T // 2      # matmul moving free dim
    TJ = T // 128    # token tiles for stats transposes
    NM = D // 256    # layer-2 output strips (256 out rows each)
    NSH1 = KI // n_cores   # w1 strips per core for the scale partial
    CW = 2048        # w2 quant chunk width (f32 elems per partition)
    NCH = (KI * 256) // CW   # w2 chunks per m strip (= 8)
    KPC = CW // 256          # k-steps per w2 chunk (= 8)
    inv_cnt = 1.0 / float(D * I)
    sqrt_d = float(np.sqrt(np.float64(D)))
    sqrt_i = float(np.sqrt(np.float64(I)))

    nc = bacc.Bacc("TRN2", num_devices=n_cores)

    xT = nc.dram_tensor("xT", [D, T], F32, kind="ExternalInput")
    w1p = nc.dram_tensor("w1p", [KI, 128, KD * 128], F32,
                         kind="ExternalInput")
    w2p = nc.dram_tensor("w2p", [NM, 128, KI * 256], F32,
                         kind="ExternalInput")
    w1sh = nc.dram_tensor("w1sh", [NSH1, 128, KD * 128], F32,
                          kind="ExternalInput")
    w2sh = nc.dram_tensor("w2sh", [128, KI * 256], F32,
                          kind="ExternalInput")
    outT = nc.dram_tensor("outT", [D, T], F32, kind="ExternalOutput")

    h_dram = nc.dram_tensor("h_scratch", [KI, 128, T], F32, kind="Internal")
    ar1_in = nc.dram_tensor("ar1_in", [8], F32, kind="Internal")
    ar1_out = nc.dram_tensor("ar1_out", [8], F32, kind="Internal",
                             addr_space="Shared")
    ar2_in = nc.dram_tensor("ar2_in", [8], F32, kind="Internal")
    ar2_out = nc.dram_tensor("ar2_out", [8], F32, kind="Internal",
                             addr_space="Shared")
    stat_dram = nc.dram_tensor("stat_dram", [6, T], F32, kind="Internal")
    srow_v = stat_dram.ap()                                       # [6, T]
    stok_v = stat_dram.ap().rearrange("r (j p) -> r p j", p=128)  # [6,128,TJ]

    xT_t = xT.ap().rearrange("(k p) t -> k p t", p=128)           # [KD,128,T]
    w1p_v = w1p.ap()
    w2p_v = w2p.ap()
    w1sh_v = w1sh.ap()
    w2sh_v = w2sh.ap()
    h_w = h_dram.ap()
    out_w = outT.ap().rearrange("(k p) t -> k p t", p=128)

    with tile.TileContext(nc) as tc:
        with (
            tc.tile_pool(name="glob", bufs=1) as glob,
            tc.tile_pool(name="psum", bufs=8, space="PSUM") as psum,
        ):
            # --- persistent constants & broadcast tiles ---
            ident = glob.tile([128, 128], F32)
            make_identity(nc, ident)
            wsc1 = glob.tile([128, 2], F32, tag="wsc1")  # cols: s1, mclip1
            wsc2 = glob.tile([128, 2], F32, tag="wsc2")
            qs1_b = glob.tile([128, T], F32, tag="qsb")
            al1_b = glob.tile([128, T], F32, tag="alb")

            with tc.tile_pool(name="stats", bufs=1) as stats:

                def part_reduce(acc, res, op):
                    # reduce [128, T] over partitions -> res [128,TJ] tok-part
                    for j in range(TJ):
                        trp = psum.tile([128, 128], F32, tag="b", name="trp")
                        nc.tensor.transpose(trp[:, :], acc[:, ts(j, 128)],
                                            ident[:, :])
                        nc.vector.tensor_reduce(
                            out=res[:, j:j + 1], in_=trp[:, :], axis=AX, op=op)

                def qs_part(Mx, r0, qs_b):
                    # qs_b = broadcast(127 / max|x| per token) [128, T]
                    qs = stats.tile([128, TJ], F32, tag="qs", name="qs")
                    nc.vector.tensor_scalar(qs, Mx, 1e-30, None, OP.max)
                    nc.vector.reciprocal(qs, qs)
                    nc.vector.tensor_scalar(qs, qs, 127.0, None, OP.mult)
                    nc.sync.dma_start(out=stok_v[r0 + 1], in_=qs[:, :])
                    qrow = stats.tile([1, T], F32, tag="qrow", name="qrow")
                    nc.sync.dma_start(out=qrow[:, :],
                                      in_=srow_v[r0 + 1:r0 + 2, :])
                    nc.gpsimd.partition_broadcast(qs_b[:, :], qrow[:, :])

                def al_part(Mx, ssq, wsc, sqrt_dim, r0, al_b):
                    # al_b = broadcast(per-token dequant scale) [128, T]
                    nrm = stats.tile([128, TJ], F32, tag="nrm", name="nrm")
                    nc.vector.tensor_scalar(nrm, ssq, 1e-38, None, OP.max)
                    nc.scalar.activation(nrm, nrm, AF.Sqrt)
                    nc.vector.tensor_scalar(nrm, nrm, 1e-12, None, OP.max)
                    inv_n = stats.tile([128, TJ], F32, tag="invn",
                                       name="inv_n")
                    nc.vector.reciprocal(inv_n, nrm)
                    al = stats.tile([128, TJ], F32, tag="al", name="al")
                    nc.vector.tensor_tensor(al, Mx, inv_n, OP.mult)
                    nc.vector.tensor_scalar(al, al, sqrt_dim, 1e-5,
                                            OP.mult, OP.max)
                    nc.vector.tensor_scalar(al, al, wsc[:, 1:2], 1.0 / 127.0,
                                            OP.mult, OP.mult)
                    nc.sync.dma_start(out=stok_v[r0 + 2], in_=al[:, :])
                    arow = stats.tile([1, T], F32, tag="arow", name="arow")
                    nc.sync.dma_start(out=arow[:, :],
                                      in_=srow_v[r0 + 2:r0 + 3, :])
                    nc.gpsimd.partition_broadcast(al_b[:, :], arow[:, :])

                def scale_trigger(wps, ar_in, ar_out):
                    # column partial sums -> scalar -> AllReduce trigger
                    wpad = stats.tile([128, 128], F32, tag="wpad",
                                      name="wpad")
                    nc.vector.memset(wpad, 0.0)
                    nc.vector.reduce_sum(wpad[:, 0:1], wps, axis=AX)
                    trw = psum.tile([128, 128], F32, tag="b", name="trw")
                    nc.tensor.transpose(trw[:, :], wpad[:, :], ident[:, :])
                    wred = stats.tile([8, 1], F32, tag="wred", name="wred")
                    nc.vector.memset(wred, 0.0)
                    nc.vector.reduce_sum(wred[0:1, :], trw[0:1, :], axis=AX)
                    nc.sync.dma_start(out=ar_in.ap()[0:8], in_=wred[:, :])
                    nc.gpsimd.collective_compute(
                        "AllReduce", OP.add,
                        replica_groups=[list(range(n_cores))],
                        ins=[ar_in.ap().opt()], outs=[ar_out.ap().opt()])

                def scale_readback(ar_out, wsc):
                    wrow = stats.tile([1, 1], F32, tag="wrow", name="wrow")
                    nc.sync.dma_start(out=wrow[:, :], in_=ar_out.ap()[0:1])
                    mrow = stats.tile([1, 2], F32, tag="mrow", name="mrow")
                    nc.vector.tensor_scalar(mrow[:, 1:2], wrow, inv_cnt,
                                            1e-5, OP.mult, OP.max)
                    nc.vector.reciprocal(mrow[:, 0:1], mrow[:, 1:2])
                    nc.gpsimd.partition_broadcast(wsc[:, :], mrow[:, :])

                with tc.tile_pool(name="bc", bufs=2) as bc:
                    # -- x strip DMAs first (kept resident in SBUF) --
                    xk_tiles = {}
                    for k in range(KD):
                        xk = bc.tile([128, T], F32, tag="xk", bufs=KD,
                                     name="xk")
                        nc.sync.dma_start(out=xk[:, :], in_=xT_t[k])
                        xk_tiles[k] = xk

                    # -- w1 scale-partial shard reads (share w1f ring) --
                    wps1 = stats.tile([128, NSH1], F32, tag="wps1",
                                      name="wps1")
                    for a in range(NSH1):
                        wtmp = bc.tile([128, KD * 128], F32, tag="w1f",
                                       bufs=4, name="wtmp")
                        nc.sync.dma_start(out=wtmp[:, :], in_=w1sh_v[a])
                        nc.vector.tensor_reduce(
                            out=wps1[:, a:a + 1], in_=wtmp[:, :], axis=AX,
                            op=OP.add, apply_absolute_value=True)

                    # -- prefetch first layer-1 weight strips --
                    w1f_tiles = {}

                    def load_w1f(i):
                        t = bc.tile([128, KD * 128], F32, tag="w1f", bufs=4,
                                    name="w1f")
                        nc.sync.dma_start(out=t[:, :], in_=w1p_v[i])
                        w1f_tiles[i] = t

                    for i in range(4):
                        load_w1f(i)

                    # ====== Stage B: x stats ======
                    am1p = stats.tile([128, T], F32, tag="amp", name="am1p")
                    am1n = stats.tile([128, T], F32, tag="amn", name="am1n")
                    sq1 = stats.tile([128, T], F32, tag="sq", name="sq1")
                    for k in range(KD):
                        xk = xk_tiles[k]
                        if k == 0:
                            nc.vector.tensor_copy(am1p, xk)
                            nc.vector.tensor_copy(am1n, xk)
                        else:
                            nc.vector.tensor_tensor(am1p, xk, am1p, OP.max)
                            nc.vector.tensor_tensor(am1n, xk, am1n, OP.min)
                        xsq = bc.tile([128, T], BF16, tag="xsq", name="xsq")
                        nc.scalar.activation(xsq, xk, AF.Square)
                        if k == 0:
                            nc.vector.tensor_copy(sq1, xsq)
                        else:
                            nc.vector.tensor_tensor(sq1, xsq, sq1, OP.add)
                    nc.vector.scalar_tensor_tensor(
                        am1n, am1n, -1.0, am1p, OP.mult, OP.max)
                    Mx1 = stats.tile([128, TJ], F32, tag="Mx", name="Mx1")
                    part_reduce(am1n, Mx1, OP.max)
                    Sq1 = stats.tile([128, TJ], F32, tag="Sq", name="Sq1")
                    part_reduce(sq1, Sq1, OP.add)
                    qs_part(Mx1, 0, qs1_b)
                    # AllReduce1: partial was reduced above; trigger + read
                    scale_trigger(wps1, ar1_in, ar1_out)
                    scale_readback(ar1_out, wsc1)
                    al_part(Mx1, Sq1, wsc1, sqrt_d, 0, al1_b)

                    # ====== Stage B2: x quant (in SBUF, no re-read) ======
                    xqT = bc.tile([128, KD, T], BF16, tag="xqT", bufs=1,
                                  name="xqT")
                    for k in range(KD):
                        xk = xk_tiles.pop(k)
                        nc.vector.tensor_tensor(xk, xk, qs1_b, OP.mult)
                        nc.vector.tensor_scalar(xqT[:, k, :], xk, C_ROUND,
                                                C_ROUND, OP.add, OP.subtract)

                    # ====== Stage C: layer 1 (+ w2 scale interleaved) ======
                    am2p = stats.tile([128, T], F32, tag="amp", name="am2p")
                    am2n = stats.tile([128, T], F32, tag="amn", name="am2n")
                    sq2 = stats.tile([128, T], F32, tag="sq", name="sq2")
                    wps2 = stats.tile([128, NCH], F32, tag="wps2",
                                      name="wps2")
                    for i in range(KI):
                        w1f = w1f_tiles.pop(i)
                        w1ff = w1f[:, :]
                        nc.scalar.activation(w1ff, w1ff, AF.Copy,
                                             scale=wsc1[:, 0:1])
                        nc.vector.tensor_scalar(w1ff, w1ff, W_CLIP, -W_CLIP,
                                                OP.min, OP.max)
                        w1q = bc.tile([128, KD, 128], BF16, tag="w1q",
                                      bufs=4, name="w1q")
                        nc.vector.tensor_scalar(
                            w1q.rearrange("p k j -> p (k j)"), w1ff, C_ROUND,
                            C_ROUND, OP.add, OP.subtract)
                        if i + 4 < KI:
                            load_w1f(i + 4)
                        hpsA = psum.tile([128, TH], F32, tag="b",
                                         name="hpsA")
                        hpsB = psum.tile([128, TH], F32, tag="b",
                                         name="hpsB")
                        for k in range(KD):
                            nc.tensor.matmul(hpsA[:, :], w1q[:, k, :],
                                             xqT[:, k, 0:TH],
                                             start=(k == 0),
                                             stop=(k == KD - 1))
                        for k in range(KD):
                            nc.tensor.matmul(hpsB[:, :], w1q[:, k, :],
                                             xqT[:, k, TH:T],
                                             start=(k == 0),
                                             stop=(k == KD - 1))
                        nc.vector.tensor_tensor(hpsA, hpsA, al1_b[:, 0:TH],
                                                OP.mult)
                        nc.vector.tensor_tensor(hpsB, hpsB, al1_b[:, TH:T],
                                                OP.mult)
                        h_sb = bc.tile([128, T], F32, tag="h", bufs=3,
                                       name="h_sb")
                        nc.scalar.activation(h_sb[:, 0:TH], hpsA, AF.Gelu)
                        nc.scalar.activation(h_sb[:, TH:T], hpsB, AF.Gelu)
                        nc.sync.dma_start(out=h_w[i], in_=h_sb[:, :])
                        if i == 0:
                            nc.vector.tensor_copy(am2p, h_sb)
                            nc.vector.tensor_copy(am2n, h_sb)
                        else:
                            nc.vector.tensor_tensor(am2p, h_sb, am2p, OP.max)
                            nc.vector.tensor_tensor(am2n, h_sb, am2n, OP.min)
                        hsq = bc.tile([128, T], BF16, tag="hsq", name="hsq")
                        nc.scalar.activation(hsq, h_sb, AF.Square)
                        if i == 0:
                            nc.vector.tensor_copy(sq2, hsq)
                        else:
                            nc.vector.tensor_tensor(sq2, hsq, sq2, OP.add)
                        # interleaved w2 scale partial (this core's strip)
                        if i < NCH:
                            wtmp2 = bc.tile([128, CW], F32, tag="w1f",
                                            bufs=4, name="wtmp2")
                            nc.sync.dma_start(
                                out=wtmp2[:, :],
                                in_=w2sh_v[:, CW * i:CW * (i + 1)])
                            nc.vector.tensor_reduce(
                                out=wps2[:, i:i + 1], in_=wtmp2[:, :],
                                axis=AX, op=OP.add,
                                apply_absolute_value=True)
                        elif i == NCH:
                            scale_trigger(wps2, ar2_in, ar2_out)
                        elif i == 56:
                            scale_readback(ar2_out, wsc2)

                    # ---- mid stats finalize ----
                    qs2_b = glob.tile([128, T], F32, tag="qsb",
                                      name="qs2_b")
                    al2_b = glob.tile([128, T], F32, tag="alb",
                                      name="al2_b")
                    nc.vector.scalar_tensor_tensor(
                        am2n, am2n, -1.0, am2p, OP.mult, OP.max)
                    Mx2 = stats.tile([128, TJ], F32, tag="Mx", name="Mx2")
                    part_reduce(am2n, Mx2, OP.max)
                    Sq2 = stats.tile([128, TJ], F32, tag="Sq", name="Sq2")
                    part_reduce(sq2, Sq2, OP.add)
                    qs_part(Mx2, 3, qs2_b)
                    al_part(Mx2, Sq2, wsc2, sqrt_i, 3, al2_b)

            # ============ Stage D: quantize h, layer 2 ============
            with tc.tile_pool(name="l2", bufs=2) as l2:
                w2qc_tiles = {}

                def issue_w2_chunk(m, q):
                    w2f = l2.tile([128, CW], F32, tag="w2f", bufs=2,
                                  name="w2f")
                    nc.sync.dma_start(
                        out=w2f[:, :],
                        in_=w2p_v[m][:, CW * q:CW * (q + 1)])
                    nc.scalar.activation(w2f[:, :], w2f[:, :], AF.Copy,
                                         scale=wsc2[:, 0:1])
                    nc.vector.tensor_scalar(w2f, w2f, W_CLIP, -W_CLIP,
                                            OP.min, OP.max)
                    t = l2.tile([128, KPC, 256], BF16, tag="w2q", bufs=8,
                                name="w2q")
                    nc.vector.tensor_scalar(
                        t.rearrange("p a c -> p (a c)"), w2f, C_ROUND,
                        C_ROUND, OP.add, OP.subtract)
                    w2qc_tiles[(m, q)] = t

                hqT = l2.tile([128, KI, T], BF16, tag="hqT", bufs=1,
                              name="hqT")
                for k2 in range(KI):
                    hk = l2.tile([128, T], F32, tag="hrd", name="hk")
                    nc.sync.dma_start(out=hk[:, :], in_=h_w[k2])
                    nc.vector.tensor_tensor(hk, hk, qs2_b, OP.mult)
                    nc.vector.tensor_scalar(hqT[:, k2, :], hk, C_ROUND,
                                            C_ROUND, OP.add, OP.subtract)
                    if k2 % KPC == KPC - 1:
                        issue_w2_chunk(0, k2 // KPC)

                for m in range(NM):
                    if m > 0:
                        for q in range(NCH):
                            issue_w2_chunk(m, q)
                    pb = [psum.tile([128, TH], F32, tag="b",
                                    name=f"pb{qq}") for qq in range(4)]
                    for kk in range(KI):
                        ch = w2qc_tiles.pop((m, kk // KPC)) \
                            if kk % KPC == KPC - 1 else \
                            w2qc_tiles[(m, kk // KPC)]
                        r = kk % KPC
                        first = (kk == 0)
                        last = (kk == KI - 1)
                        nc.tensor.matmul(pb[0][:, :], ch[:, r, 0:128],
                                         hqT[:, kk, 0:TH],
                                         start=first, stop=last)
                        nc.tensor.matmul(pb[1][:, :], ch[:, r, 0:128],
                                         hqT[:, kk, TH:T],
                                         start=first, stop=last)
                        nc.tensor.matmul(pb[2][:, :], ch[:, r, 128:256],
                                         hqT[:, kk, 0:TH],
                                         start=first, stop=last)
                        nc.tensor.matmul(pb[3][:, :], ch[:, r, 128:256],
                                         hqT[:, kk, TH:T],
                                         start=first, stop=last)
                    for qq in range(4):
                        jcol = qq >> 1
                        half = qq & 1
                        ob = l2.tile([128, TH], F32, tag="ob", bufs=3,
                                     name="ob")
                        nc.vector.tensor_tensor(
                            ob, pb[qq], al2_b[:, ts(half, TH)], OP.mult)
                        nc.sync.dma_start(
                            out=out_w[2 * m + jcol][:, ts(half, TH)],
                            in_=ob[:, :])

    nc.compile()  # Bacc passes: EVSEM multi-wait lowering, library loads,
    return nc     # extended-ISA codegen, nop fusion, register alloc


_NC_CACHE = {}


def _get_nc(D, I, T, n_cores):
    key = (D, I, T, n_cores)
    if key not in _NC_CACHE:
        _NC_CACHE[key] = build_bitnet(D, I, T, n_cores)
    return _NC_CACHE[key]


def make_in_maps(x, w1, w2, n_cores=N_CORES):
    """Host-side sharding/layout only (transpose + slicing, no arithmetic)."""
    xf = np.ascontiguousarray(np.asarray(x, dtype=np.float32)).reshape(
        -1, x.shape[-1])
    D = xf.shape[1]
    I = w1.shape[0]
    T = xf.shape[0] // n_cores
    KD = D // 128
    KI = I // 128
    NM = D // 256
    w1np = np.asarray(w1, dtype=np.float32)
    w2np = np.asarray(w2, dtype=np.float32)
    # strip-major stationary layouts (pure element permutations)
    # w1p[s, p, k*128+j] = w1[128s+j, 128k+p]
    w1p = np.ascontiguousarray(
        w1np.reshape(KI, 128, KD, 128).transpose(0, 3, 2, 1)).reshape(
        KI, 128, KD * 128)
    # w2p[m, p, k*256+c] = w2[256m+c, 128k+p]
    w2p = np.ascontiguousarray(
        w2np.reshape(NM, 256, KI, 128).transpose(0, 3, 2, 1)).reshape(
        NM, 128, KI * 256)
    in_maps = []
    for c in range(n_cores):
        xTc = np.ascontiguousarray(xf[c * T:(c + 1) * T].T)  # [D, T]
        in_maps.append({
            "xT": xTc,
            "w1p": w1p,
            "w2p": w2p,
            "w1sh": w1p[c * (KI // n_cores):(c + 1) * (KI // n_cores)],
            "w2sh": w2p[c],
        })
    return in_maps, (D, I, T)


def run_spmd(x, w1, w2, trace=False, **kwargs):
    from concourse.bass_utils import run_bass_kernel_spmd

    B, S, D = x.shape
    in_maps, (D, I, T) = make_in_maps(x, w1, w2, N_CORES)
    nc = _get_nc(D, I, T, N_CORES)
    res = run_bass_kernel_spmd(nc, in_maps, core_ids=list(range(N_CORES)),
                               trace=trace, **kwargs)
    outs = [res.results[c]["outT"].T for c in range(N_CORES)]  # each [T, D]
    out = np.concatenate(outs, axis=0).reshape(B, S, D)
    return np.ascontiguousarray(out, dtype=np.float32), res

def kernel(x, w1, w2):
    out, _ = run_spmd(x, w1, w2, trace=False)
    return out


# revision 7
# speedup vs baseline: 2.8881x; 1.0206x over previous
"""BitNet FFN (bitlinear158 -> gelu -> bitlinear158) Trainium2 kernel.

Sharding: data-parallel over tokens across 8 cores (1024 tokens/core).
Layout: tokens on the free axis everywhere; weights stationary in the PE.

v3 design:
  - No weight AllGathers: each core streams the FULL weights from its own
    DRAM copy in strip-major contiguous layout (8KB DMA rows) and
    quantizes strips locally, pipelined under the matmuls.
  - Only collective: one tiny AllReduce per layer for the weight-scale
    partial sum (mean|W| must be global-exact); trigger issued early,
    result readback deferred so no engine queue blocks on it.
  - x strips stay resident in SBUF between the stats pass and the quant
    pass (no second HBM read of x).
  - All round-to-int ops (C_ROUND add/sub) run on GPSIMD; clips/scales
    on vector, squares/gelu/weight-scale on scalar: every engine stays
    below the PE's per-strip budget.
  - Layer-2 h quantization interleaved with m=0's weight chunks so the
    PE k-accumulation of the first output group consumes hqT strips as
    they are produced.

Math notes (exactness, same recipe as baseline):
  - activation quant ints = round(x * 127 / max|x|)  (the rms-norm cancels)
  - weight quant ternary = clip(round(w / clip(mean|w|,1e-5)), -1, 1)
  - both exactly representable in bf16; PSUM accumulates integer products
    (<= 2^21) exactly in fp32, so the matmuls are exact.
  - per-token output scale alpha = clip(max|x|*sqrt(d)/||x||, 1e-5)
      * clip(mean|w|,1e-5) / 127 applied on PSUM before gelu.
  - round-to-nearest-even via fp32 (t + 1.5*2**23) - 1.5*2**23, matching
    jnp.round; clip(round(t),-1,1) == round(clamp(t, +-1.4999999)).
"""

import sys

for _p in ("/opt/trn_rl_repo", "/opt/trn_rl_repo/concourse"):
    if _p not in sys.path:
        sys.path.insert(0, _p)

import numpy as np

import concourse.bass as bass
import concourse.bacc as bacc
import concourse.mybir as mybir
import concourse.tile as tile
from concourse import library_config
from concourse.bass import ts
from concourse.masks import make_identity

F32 = mybir.dt.float32
BF16 = mybir.dt.bfloat16
AX = mybir.AxisListType.X
OP = mybir.AluOpType
AF = mybir.ActivationFunctionType

C_ROUND = 12582912.0  # 1.5 * 2**23 : fp32 RNE rounding constant
W_CLIP = 1.4999999    # round(clamp(t, +-W_CLIP)) == clip(round(t), -1, 1)
N_CORES = 8


def build_bitnet(D, I, T, n_cores=N_CORES):
    """Per-core SPMD Bass program.

    Per-core I/O: xT [D,T] f32 (token shard, transposed); w1p
    [I/128,128,(D/128)*128] f32 and w2p [D/256,128,(I/128)*256] f32
    (full weights, strip-major); w1sh/w2sh (this core's 1/n slice of
    each, for the mean|w| partial) -> outT [D,T] f32.
    """
    KD = D // 128    # d tiles (layer-1 contraction; layer-2 output side)
    KI = I // 128    # inner tiles
    TH = T // 2      # matmul moving free dim
    TJ = T // 128    # token tiles for stats transposes
    NM = D // 256    # layer-2 output strips (256 out rows each)
    NSH1 = KI // n_cores   # w1 strips per core for the scale partial
    CW = 2048        # w2 quant chunk width (f32 elems per partition)
    NCH = (KI * 256) // CW   # w2 chunks per m strip (= 8)
    KPC = CW // 256          # k-steps per w2 chunk (= 8)
    inv_cnt = 1.0 / float(D * I)
    sqrt_d = float(np.sqrt(np.float64(D)))
    sqrt_i = float(np.sqrt(np.float64(I)))

    nc = bacc.Bacc("TRN2", num_devices=n_cores)

    xT = nc.dram_tensor("xT", [D, T], F32, kind="ExternalInput")
    w1p = nc.dram_tensor("w1p", [KI, 128, KD * 128], F32,
                         kind="ExternalInput")
    w2p = nc.dram_tensor("w2p", [NM, 128, KI * 256], F32,
                         kind="ExternalInput")
    w1sh = nc.dram_tensor("w1sh", [NSH1, 128, KD * 128], F32,
                          kind="ExternalInput")
    w2sh = nc.dram_tensor("w2sh", [128, KI * 256], F32,
                          kind="ExternalInput")
    outT = nc.dram_tensor("outT", [D, T], F32, kind="ExternalOutput")

    h_dram = nc.dram_tensor("h_scratch", [KI, 128, T], F32, kind="Internal")
    ar1_in = nc.dram_tensor("ar1_in", [8], F32, kind="Internal")
    ar1_out = nc.dram_tensor("ar1_out", [8], F32, kind="Internal",
                             addr_space="Shared")
    ar2_in = nc.dram_tensor("ar2_in", [8], F32, kind="Internal")
    ar2_out = nc.dram_tensor("ar2_out", [8], F32, kind="Internal",
                             addr_space="Shared")
    stat_dram = nc.dram_tensor("stat_dram", [6, T], F32, kind="Internal")
    srow_v = stat_dram.ap()                                       # [6, T]
    stok_v = stat_dram.ap().rearrange("r (j p) -> r p j", p=128)  # [6,128,TJ]

    xT_t = xT.ap().rearrange("(k p) t -> k p t", p=128)           # [KD,128,T]
    w1p_v = w1p.ap()
    w2p_v = w2p.ap()
    w1sh_v = w1sh.ap()
    w2sh_v = w2sh.ap()
    h_w = h_dram.ap()
    out_w = outT.ap().rearrange("(k p) t -> k p t", p=128)

    with tile.TileContext(nc) as tc:
        with (
            tc.tile_pool(name="glob", bufs=1) as glob,
            tc.tile_pool(name="psum", bufs=8, space="PSUM") as psum,
        ):
            # --- persistent constants & broadcast tiles ---
            ident = glob.tile([128, 128], F32)
            make_identity(nc, ident)
            wsc1 = glob.tile([128, 2], F32, tag="wsc1")  # cols: s1, mclip1
            wsc2 = glob.tile([128, 2], F32, tag="wsc2")
            qs1_b = glob.tile([128, T], F32, tag="qsb")
            al1_b = glob.tile([128, T], F32, tag="alb")

            with tc.tile_pool(name="stats", bufs=1) as stats:

                def part_reduce(acc, res, op):
                    # reduce [128, T] over partitions -> res [128,TJ] tok-part
                    for j in range(TJ):
                        trp = psum.tile([128, 128], F32, tag="b", name="trp")
                        nc.tensor.transpose(trp[:, :], acc[:, ts(j, 128)],
                                            ident[:, :])
                        nc.vector.tensor_reduce(
                            out=res[:, j:j + 1], in_=trp[:, :], axis=AX, op=op)

                def qs_part(Mx, r0, qs_b):
                    # qs_b = broadcast(127 / max|x| per token) [128, T]
                    qs = stats.tile([128, TJ], F32, tag="qs", name="qs")
                    nc.vector.tensor_scalar(qs, Mx, 1e-30, None, OP.max)
                    nc.vector.reciprocal(qs, qs)
                    nc.vector.tensor_scalar(qs, qs, 127.0, None, OP.mult)
                    nc.sync.dma_start(out=stok_v[r0 + 1], in_=qs[:, :])
                    qrow = stats.tile([1, T], F32, tag="qrow", name="qrow")
                    nc.sync.dma_start(out=qrow[:, :],
                                      in_=srow_v[r0 + 1:r0 + 2, :])
                    nc.gpsimd.partition_broadcast(qs_b[:, :], qrow[:, :])

                def al_part(Mx, ssq, wsc, sqrt_dim, r0, al_b):
                    # al_b = broadcast(per-token dequant scale) [128, T]
                    nrm = stats.tile([128, TJ], F32, tag="nrm", name="nrm")
                    nc.vector.tensor_scalar(nrm, ssq, 1e-38, None, OP.max)
                    nc.scalar.activation(nrm, nrm, AF.Sqrt)
                    nc.vector.tensor_scalar(nrm, nrm, 1e-12, None, OP.max)
                    inv_n = stats.tile([128, TJ], F32, tag="invn",
                                       name="inv_n")
                    nc.vector.reciprocal(inv_n, nrm)
                    al = stats.tile([128, TJ], F32, tag="al", name="al")
                    nc.vector.tensor_tensor(al, Mx, inv_n, OP.mult)
                    nc.vector.tensor_scalar(al, al, sqrt_dim, 1e-5,
                                            OP.mult, OP.max)
                    nc.vector.tensor_scalar(al, al, wsc[:, 1:2], 1.0 / 127.0,
                                            OP.mult, OP.mult)
                    nc.sync.dma_start(out=stok_v[r0 + 2], in_=al[:, :])
                    arow = stats.tile([1, T], F32, tag="arow", name="arow")
                    nc.sync.dma_start(out=arow[:, :],
                                      in_=srow_v[r0 + 2:r0 + 3, :])
                    nc.gpsimd.partition_broadcast(al_b[:, :], arow[:, :])

                def scale_trigger(wps, ar_in, ar_out):
                    # column partial sums -> scalar -> AllReduce trigger
                    wpad = stats.tile([128, 128], F32, tag="wpad",
                                      name="wpad")
                    nc.vector.memset(wpad, 0.0)
                    nc.vector.reduce_sum(wpad[:, 0:1], wps, axis=AX)
                    trw = psum.tile([128, 128], F32, tag="b", name="trw")
                    nc.tensor.transpose(trw[:, :], wpad[:, :], ident[:, :])
                    wred = stats.tile([8, 1], F32, tag="wred", name="wred")
                    nc.vector.memset(wred, 0.0)
                    nc.vector.reduce_sum(wred[0:1, :], trw[0:1, :], axis=AX)
                    nc.sync.dma_start(out=ar_in.ap()[0:8], in_=wred[:, :])
                    nc.gpsimd.collective_compute(
                        "AllReduce", OP.add,
                        replica_groups=[list(range(n_cores))],
                        ins=[ar_in.ap().opt()], outs=[ar_out.ap().opt()])

                def scale_readback(ar_out, wsc):
                    wrow = stats.tile([1, 1], F32, tag="wrow", name="wrow")
                    nc.sync.dma_start(out=wrow[:, :], in_=ar_out.ap()[0:1])
                    mrow = stats.tile([1, 2], F32, tag="mrow", name="mrow")
                    nc.vector.tensor_scalar(mrow[:, 1:2], wrow, inv_cnt,
                                            1e-5, OP.mult, OP.max)
                    nc.vector.reciprocal(mrow[:, 0:1], mrow[:, 1:2])
                    nc.gpsimd.partition_broadcast(wsc[:, :], mrow[:, :])

                with tc.tile_pool(name="bc", bufs=2) as bc:
                    # -- w1 scale-partial shard reads first: the AllReduce
                    #    rendezvous is the longest dependency chain --
                    wps1 = stats.tile([128, NSH1], F32, tag="wps1",
                                      name="wps1")
                    for a in range(NSH1):
                        wtmp = bc.tile([128, KD * 128], F32, tag="w1f",
                                       bufs=4, name="wtmp")
                        nc.sync.dma_start(out=wtmp[:, :], in_=w1sh_v[a])
                        nc.vector.tensor_reduce(
                            out=wps1[:, a:a + 1], in_=wtmp[:, :], axis=AX,
                            op=OP.add, apply_absolute_value=True)
                    scale_trigger(wps1, ar1_in, ar1_out)

                    # -- x strip DMAs (kept resident in SBUF) --
                    xk_tiles = {}
                    for k in range(KD):
                        xk = bc.tile([128, T], F32, tag="xk", bufs=KD,
                                     name="xk")
                        nc.sync.dma_start(out=xk[:, :], in_=xT_t[k])
                        xk_tiles[k] = xk

                    # -- prefetch first layer-1 weight strips --
                    w1f_tiles = {}

                    def load_w1f(i):
                        t = bc.tile([128, KD * 128], F32, tag="w1f", bufs=4,
                                    name="w1f")
                        nc.sync.dma_start(out=t[:, :], in_=w1p_v[i])
                        w1f_tiles[i] = t

                    for i in range(4):
                        load_w1f(i)

                    # ====== Stage B: x stats ======
                    am1p = stats.tile([128, T], F32, tag="amp", name="am1p")
                    am1n = stats.tile([128, T], F32, tag="amn", name="am1n")
                    sq1 = stats.tile([128, T], F32, tag="sq", name="sq1")
                    for k in range(KD):
                        xk = xk_tiles[k]
                        if k == 0:
                            nc.vector.tensor_copy(am1p, xk)
                            nc.vector.tensor_copy(am1n, xk)
                        else:
                            nc.vector.tensor_tensor(am1p, xk, am1p, OP.max)
                            nc.vector.tensor_tensor(am1n, xk, am1n, OP.min)
                        xsq = bc.tile([128, T], BF16, tag="xsq", name="xsq")
                        nc.scalar.activation(xsq, xk, AF.Square)
                        if k == 0:
                            nc.vector.tensor_copy(sq1, xsq)
                        else:
                            nc.vector.tensor_tensor(sq1, xsq, sq1, OP.add)
                    nc.vector.scalar_tensor_tensor(
                        am1n, am1n, -1.0, am1p, OP.mult, OP.max)
                    Mx1 = stats.tile([128, TJ], F32, tag="Mx", name="Mx1")
                    part_reduce(am1n, Mx1, OP.max)
                    Sq1 = stats.tile([128, TJ], F32, tag="Sq", name="Sq1")
                    part_reduce(sq1, Sq1, OP.add)
                    qs_part(Mx1, 0, qs1_b)
                    scale_readback(ar1_out, wsc1)
                    al_part(Mx1, Sq1, wsc1, sqrt_d, 0, al1_b)

                    # ====== Stage B2: x quant (in SBUF, no re-read) ======
                    xqT = bc.tile([128, KD, T], BF16, tag="xqT", bufs=1,
                                  name="xqT")
                    for k in range(KD):
                        nc.vector.tensor_tensor(xk_tiles[k], xk_tiles[k],
                                                qs1_b, OP.mult)
                    for k in range(KD):
                        nc.scalar.activation(xk_tiles[k], xk_tiles[k],
                                             AF.Copy, bias=C_ROUND)
                    for k in range(KD):
                        xk = xk_tiles.pop(k)
                        nc.vector.tensor_scalar(xqT[:, k, :], xk, C_ROUND,
                                                None, OP.subtract)

                    # ====== Stage C: layer 1 (+ w2 scale interleaved) ======
                    am2p = stats.tile([128, T], F32, tag="amp", name="am2p")
                    am2n = stats.tile([128, T], F32, tag="amn", name="am2n")
                    sq2 = stats.tile([128, T], F32, tag="sq", name="sq2")
                    wps2 = stats.tile([128, NCH], F32, tag="wps2",
                                      name="wps2")
                    w1q_tiles = {}

                    def quant_w1(i):
                        # ternary-quantize strip i (consumes w1f_tiles[i])
                        w1f = w1f_tiles.pop(i)
                        w1ff = w1f[:, :]
                        nc.scalar.activation(w1ff, w1ff, AF.Copy,
                                             scale=wsc1[:, 0:1])
                        nc.vector.tensor_scalar(w1ff, w1ff, W_CLIP, -W_CLIP,
                                                OP.min, OP.max)
                        w1q = bc.tile([128, KD, 128], BF16, tag="w1q",
                                      bufs=4, name="w1q")
                        nc.vector.tensor_scalar(
                            w1q.rearrange("p k j -> p (k j)"), w1ff, C_ROUND,
                            C_ROUND, OP.add, OP.subtract)
                        w1q_tiles[i] = w1q

                    quant_w1(0)
                    for i in range(KI):
                        # quantize NEXT strip first: keeps the scalar/vector
                        # queues from blocking strip i+1's weights behind
                        # strip i's gelu/stats (in-order queues)
                        if i + 1 < KI:
                            quant_w1(i + 1)
                        if i + 4 < KI:
                            load_w1f(i + 4)
                        w1q = w1q_tiles.pop(i)
                        hpsA = psum.tile([128, TH], F32, tag="b",
                                         name="hpsA")
                        hpsB = psum.tile([128, TH], F32, tag="b",
                                         name="hpsB")
                        for k in range(KD):
                            nc.tensor.matmul(hpsA[:, :], w1q[:, k, :],
                                             xqT[:, k, 0:TH],
                                             start=(k == 0),
                                             stop=(k == KD - 1))
                        for k in range(KD):
                            nc.tensor.matmul(hpsB[:, :], w1q[:, k, :],
                                             xqT[:, k, TH:T],
                                             start=(k == 0),
                                             stop=(k == KD - 1))
                        nc.vector.tensor_tensor(hpsA, hpsA, al1_b[:, 0:TH],
                                                OP.mult)
                        nc.vector.tensor_tensor(hpsB, hpsB, al1_b[:, TH:T],
                                                OP.mult)
                        h_sb = bc.tile([128, T], F32, tag="h", bufs=3,
                                       name="h_sb")
                        nc.scalar.activation(h_sb[:, 0:TH], hpsA, AF.Gelu)
                        nc.scalar.activation(h_sb[:, TH:T], hpsB, AF.Gelu)
                        nc.sync.dma_start(out=h_w[i], in_=h_sb[:, :])
                        if i == 0:
                            nc.vector.tensor_copy(am2p, h_sb)
                            nc.vector.tensor_copy(am2n, h_sb)
                        else:
                            nc.vector.tensor_tensor(am2p, h_sb, am2p, OP.max)
                            nc.vector.tensor_tensor(am2n, h_sb, am2n, OP.min)
                        hsq = bc.tile([128, T], BF16, tag="hsq", name="hsq")
                        nc.scalar.activation(hsq, h_sb, AF.Square)
                        if i == 0:
                            nc.vector.tensor_copy(sq2, hsq)
                        else:
                            nc.vector.tensor_tensor(sq2, hsq, sq2, OP.add)
                        # interleaved w2 scale partial (this core's strip)
                        if i < NCH:
                            wtmp2 = bc.tile([128, CW], F32, tag="w1f",
                                            bufs=4, name="wtmp2")
                            nc.sync.dma_start(
                                out=wtmp2[:, :],
                                in_=w2sh_v[:, CW * i:CW * (i + 1)])
                            nc.vector.tensor_reduce(
                                out=wps2[:, i:i + 1], in_=wtmp2[:, :],
                                axis=AX, op=OP.add,
                                apply_absolute_value=True)
                        elif i == NCH:
                            scale_trigger(wps2, ar2_in, ar2_out)
                        elif i == 56:
                            scale_readback(ar2_out, wsc2)

                    # ---- mid stats finalize ----
                    qs2_b = glob.tile([128, T], F32, tag="qsb",
                                      name="qs2_b")
                    al2_b = glob.tile([128, T], F32, tag="alb",
                                      name="al2_b")
                    nc.vector.scalar_tensor_tensor(
                        am2n, am2n, -1.0, am2p, OP.mult, OP.max)
                    Mx2 = stats.tile([128, TJ], F32, tag="Mx", name="Mx2")
                    part_reduce(am2n, Mx2, OP.max)
                    Sq2 = stats.tile([128, TJ], F32, tag="Sq", name="Sq2")
                    part_reduce(sq2, Sq2, OP.add)
                    qs_part(Mx2, 3, qs2_b)
                    al_part(Mx2, Sq2, wsc2, sqrt_i, 3, al2_b)

            # ============ Stage D: quantize h, layer 2 ============
            with tc.tile_pool(name="l2", bufs=2) as l2:
                w2qc_tiles = {}

                def issue_w2_chunk(m, q):
                    w2f = l2.tile([128, CW], F32, tag="w2f", bufs=2,
                                  name="w2f")
                    nc.sync.dma_start(
                        out=w2f[:, :],
                        in_=w2p_v[m][:, CW * q:CW * (q + 1)])
                    nc.scalar.activation(w2f[:, :], w2f[:, :], AF.Copy,
                                         scale=wsc2[:, 0:1])
                    nc.vector.tensor_scalar(w2f, w2f, W_CLIP, -W_CLIP,
                                            OP.min, OP.max)
                    t = l2.tile([128, KPC, 256], BF16, tag="w2q", bufs=8,
                                name="w2q")
                    nc.vector.tensor_scalar(
                        t.rearrange("p a c -> p (a c)"), w2f, C_ROUND,
                        C_ROUND, OP.add, OP.subtract)
                    w2qc_tiles[(m, q)] = t

                hqT = l2.tile([128, KI, T], BF16, tag="hqT", bufs=1,
                              name="hqT")
                for k2 in range(KI):
                    hk = l2.tile([128, T], F32, tag="hrd", name="hk")
                    nc.sync.dma_start(out=hk[:, :], in_=h_w[k2])
                    nc.vector.tensor_tensor(hk, hk, qs2_b, OP.mult)
                    nc.vector.tensor_scalar(hqT[:, k2, :], hk, C_ROUND,
                                            C_ROUND, OP.add, OP.subtract)
                    if k2 % KPC == KPC - 1:
                        issue_w2_chunk(0, k2 // KPC)

                for m in range(NM):
                    # prefetch NEXT group's chunks before this group's
                    # matmuls/output ops so their quant never queues behind
                    # ob-scale ops that wait on this group's PSUM
                    if m + 1 < NM:
                        for q in range(NCH):
                            issue_w2_chunk(m + 1, q)
                    pb = [psum.tile([128, TH], F32, tag="b",
                                    name=f"pb{qq}") for qq in range(4)]
                    for kk in range(KI):
                        ch = w2qc_tiles.pop((m, kk // KPC)) \
                            if kk % KPC == KPC - 1 else \
                            w2qc_tiles[(m, kk // KPC)]
                        r = kk % KPC
                        first = (kk == 0)
                        last = (kk == KI - 1)
                        nc.tensor.matmul(pb[0][:, :], ch[:, r, 0:128],
                                         hqT[:, kk, 0:TH],
                                         start=first, stop=last)
                        nc.tensor.matmul(pb[1][:, :], ch[:, r, 0:128],
                                         hqT[:, kk, TH:T],
                                         start=first, stop=last)
                        nc.tensor.matmul(pb[2][:, :], ch[:, r, 128:256],
                                         hqT[:, kk, 0:TH],
                                         start=first, stop=last)
                        nc.tensor.matmul(pb[3][:, :], ch[:, r, 128:256],
                                         hqT[:, kk, TH:T],
                                         start=first, stop=last)
                    for qq in range(4):
                        jcol = qq >> 1
                        half = qq & 1
                        ob = l2.tile([128, TH], F32, tag="ob", bufs=2,
                                     name="ob")
                        nc.vector.tensor_tensor(
                            ob, pb[qq], al2_b[:, ts(half, TH)], OP.mult)
                        nc.sync.dma_start(
                            out=out_w[2 * m + jcol][:, ts(half, TH)],
                            in_=ob[:, :])

    nc.compile()  # Bacc passes: EVSEM multi-wait lowering, library loads,
    return nc     # extended-ISA codegen, nop fusion, register alloc


_NC_CACHE = {}


def _get_nc(D, I, T, n_cores):
    key = (D, I, T, n_cores)
    if key not in _NC_CACHE:
        _NC_CACHE[key] = build_bitnet(D, I, T, n_cores)
    return _NC_CACHE[key]


def make_in_maps(x, w1, w2, n_cores=N_CORES):
    """Host-side sharding/layout only (transpose + slicing, no arithmetic)."""
    xf = np.ascontiguousarray(np.asarray(x, dtype=np.float32)).reshape(
        -1, x.shape[-1])
    D = xf.shape[1]
    I = w1.shape[0]
    T = xf.shape[0] // n_cores
    KD = D // 128
    KI = I // 128
    NM = D // 256
    w1np = np.asarray(w1, dtype=np.float32)
    w2np = np.asarray(w2, dtype=np.float32)
    # strip-major stationary layouts (pure element permutations)
    # w1p[s, p, k*128+j] = w1[128s+j, 128k+p]
    w1p = np.ascontiguousarray(
        w1np.reshape(KI, 128, KD, 128).transpose(0, 3, 2, 1)).reshape(
        KI, 128, KD * 128)
    # w2p[m, p, k*256+c] = w2[256m+c, 128k+p]
    w2p = np.ascontiguousarray(
        w2np.reshape(NM, 256, KI, 128).transpose(0, 3, 2, 1)).reshape(
        NM, 128, KI * 256)
    in_maps = []
    for c in range(n_cores):
        xTc = np.ascontiguousarray(xf[c * T:(c + 1) * T].T)  # [D, T]
        in_maps.append({
            "xT": xTc,
            "w1p": w1p,
            "w2p": w2p,
            "w1sh": w1p[c * (KI // n_cores):(c + 1) * (KI // n_cores)],
            "w2sh": w2p[c],
        })
    return in_maps, (D, I, T)


def run_spmd(x, w1, w2, trace=False, **kwargs):
    from concourse.bass_utils import run_bass_kernel_spmd

    B, S, D = x.shape
    in_maps, (D, I, T) = make_in_maps(x, w1, w2, N_CORES)
    nc = _get_nc(D, I, T, N_CORES)
    res = run_bass_kernel_spmd(nc, in_maps, core_ids=list(range(N_CORES)),
                               trace=trace, **kwargs)
    outs = [res.results[c]["outT"].T for c in range(N_CORES)]  # each [T, D]
    out = np.concatenate(outs, axis=0).reshape(B, S, D)
    return np.ascontiguousarray(out, dtype=np.float32), res

def kernel(x, w1, w2):
    out, _ = run_spmd(x, w1, w2, trace=False)
    return out


# revision 10
# speedup vs baseline: 2.9762x; 1.0305x over previous
"""BitNet FFN (bitlinear158 -> gelu -> bitlinear158) Trainium2 kernel.

Sharding: data-parallel over tokens across 8 cores (1024 tokens/core).
Layout: tokens on the free axis everywhere; weights stationary in the PE.

v6 design:
  - No weight AllGathers: each core streams the FULL weights from its own
    DRAM copy in strip-major contiguous layout (8KB DMA rows) and
    quantizes strips locally, pipelined under the matmuls.
  - Only collective: one tiny AllReduce per layer for the weight-scale
    partial (mean|W| must be global-exact). Its input DMA + trigger are
    issued before any bulk x/weight DMA so the rendezvous completes
    during the x-stats pass; scale broadcasts that gate compute go
    through PE ones-matmuls (the gpsimd queue blocks on collectives).
  - x strips stay resident in SBUF between stats and quant (no re-read).
  - Stats engine split: absmax = scalar Abs + single vector max chain;
    sum-of-squares for layer 1 = PE ones-matmul accumulation into a
    [1,T] psum row (no vector adds, no transpose pass).
  - Layer-2: h-read pipeline 5 deep; output groups m=0 and m=1 issue
    their k-accumulation interleaved so the PE consumes hqT strips at
    the vector engine's production rate with no first-group stall.

Math notes (exactness, same recipe as baseline):
  - activation quant ints = round(x * 127 / max|x|)  (the rms-norm cancels)
  - weight quant ternary = clip(round(w / clip(mean|w|,1e-5)), -1, 1)
  - both exactly representable in bf16; PSUM accumulates integer products
    (<= 2^21) exactly in fp32, so the matmuls are exact.
  - per-token output scale alpha = clip(max|x|*sqrt(d)/||x||, 1e-5)
      * clip(mean|w|,1e-5) / 127 applied on PSUM before gelu.
  - round-to-nearest-even via fp32 (t + 1.5*2**23) - 1.5*2**23, matching
    jnp.round; clip(round(t),-1,1) == round(clamp(t, +-1.4999999)).
"""

import sys

for _p in ("/opt/trn_rl_repo", "/opt/trn_rl_repo/concourse"):
    if _p not in sys.path:
        sys.path.insert(0, _p)

import numpy as np

import concourse.bass as bass
import concourse.bacc as bacc
import concourse.mybir as mybir
import concourse.tile as tile
from concourse import library_config
from concourse.bass import ts
from concourse.masks import make_identity

F32 = mybir.dt.float32
BF16 = mybir.dt.bfloat16
AX = mybir.AxisListType.X
OP = mybir.AluOpType
AF = mybir.ActivationFunctionType

C_ROUND = 12582912.0  # 1.5 * 2**23 : fp32 RNE rounding constant
W_CLIP = 1.4999999    # round(clamp(t, +-W_CLIP)) == clip(round(t), -1, 1)
N_CORES = 8


def build_bitnet(D, I, T, n_cores=N_CORES):
    """Per-core SPMD Bass program.

    Per-core I/O: xT [D,T] f32 (token shard, transposed); w1p
    [I/128,128,(D/128)*128] f32 and w2p [D/256,128,(I/128)*256] f32
    (full weights, strip-major); w1sh/w2sh (this core's 1/n slice of
    each, for the mean|w| partial) -> outT [D,T] f32.
    """
    KD = D // 128    # d tiles (layer-1 contraction; layer-2 output side)
    KI = I // 128    # inner tiles
    TH = T // 2      # matmul moving free dim
    TJ = T // 128    # token tiles for stats transposes
    NM = D // 256    # layer-2 output strips (256 out rows each)
    NSH1 = KI // n_cores   # w1 strips per core for the scale partial
    CW = 2048        # w2 quant chunk width (f32 elems per partition)
    NCH = (KI * 256) // CW   # w2 chunks per m strip (= 8)
    KPC = CW // 256          # k-steps per w2 chunk (= 8)
    inv_cnt = 1.0 / float(D * I)
    sqrt_d = float(np.sqrt(np.float64(D)))
    sqrt_i = float(np.sqrt(np.float64(I)))

    nc = bacc.Bacc("TRN2", num_devices=n_cores)

    xT = nc.dram_tensor("xT", [D, T], F32, kind="ExternalInput")
    w1p = nc.dram_tensor("w1p", [KI, 128, KD * 128], F32,
                         kind="ExternalInput")
    w2p = nc.dram_tensor("w2p", [NM, 128, KI * 256], F32,
                         kind="ExternalInput")
    w1sh = nc.dram_tensor("w1sh", [NSH1, 128, KD * 128], F32,
                          kind="ExternalInput")
    w2sh = nc.dram_tensor("w2sh", [128, KI * 256], F32,
                          kind="ExternalInput")
    outT = nc.dram_tensor("outT", [D, T], F32, kind="ExternalOutput")

    h_dram = nc.dram_tensor("h_scratch", [KI, 128, T], F32, kind="Internal")
    ar1_in = nc.dram_tensor("ar1_in", [8], F32, kind="Internal")
    ar1_out = nc.dram_tensor("ar1_out", [8], F32, kind="Internal",
                             addr_space="Shared")
    ar2_in = nc.dram_tensor("ar2_in", [8], F32, kind="Internal")
    ar2_out = nc.dram_tensor("ar2_out", [8], F32, kind="Internal",
                             addr_space="Shared")
    stat_dram = nc.dram_tensor("stat_dram", [6, T], F32, kind="Internal")
    srow_v = stat_dram.ap()                                       # [6, T]
    stok_v = stat_dram.ap().rearrange("r (j p) -> r p j", p=128)  # [6,128,TJ]

    xT_t = xT.ap().rearrange("(k p) t -> k p t", p=128)           # [KD,128,T]
    w1p_v = w1p.ap()
    w2p_v = w2p.ap()
    w1sh_v = w1sh.ap()
    w2sh_v = w2sh.ap()
    h_w = h_dram.ap()
    out_w = outT.ap().rearrange("(k p) t -> k p t", p=128)

    with tile.TileContext(nc) as tc:
        with (
            tc.tile_pool(name="glob", bufs=1) as glob,
            tc.tile_pool(name="psum", bufs=8, space="PSUM") as psum,
        ):
            # --- persistent constants & broadcast tiles ---
            ident = glob.tile([128, 128], F32)
            make_identity(nc, ident)
            ones_c = glob.tile([128, 1], BF16, tag="ones_c")
            nc.vector.memset(ones_c, 1.0)
            ones_r = glob.tile([1, 128], F32, tag="ones_r")
            nc.vector.memset(ones_r, 1.0)
            wsc1 = glob.tile([128, 2], F32, tag="wsc1")  # cols: s1, mclip1
            wsc2 = glob.tile([128, 2], F32, tag="wsc2")
            qs1_b = glob.tile([128, T], F32, tag="qsb")
            al1_b = glob.tile([128, T], F32, tag="alb")

            with tc.tile_pool(name="stats", bufs=1) as stats:

                def part_reduce(acc, res, op):
                    # reduce [128, T] over partitions -> res [128,TJ] tok-part
                    for j in range(TJ):
                        trp = psum.tile([128, 128], F32, tag="b", name="trp")
                        nc.tensor.transpose(trp[:, :], acc[:, ts(j, 128)],
                                            ident[:, :])
                        nc.vector.tensor_reduce(
                            out=res[:, j:j + 1], in_=trp[:, :], axis=AX, op=op)

                def qs_part(Mx, r0, qs_b):
                    # writes Mx row (stok r0) + qs row (stok r0+1), reads the
                    # qs row back and broadcasts via PE (gpsimd may be
                    # blocked on a collective). Returns the Mx row tile.
                    nc.sync.dma_start(out=stok_v[r0], in_=Mx[:, :])
                    qs = stats.tile([128, TJ], F32, tag="qs", name="qs")
                    nc.vector.tensor_scalar(qs, Mx, 1e-30, None, OP.max)
                    nc.vector.reciprocal(qs, qs)
                    nc.vector.tensor_scalar(qs, qs, 127.0, None, OP.mult)
                    nc.sync.dma_start(out=stok_v[r0 + 1], in_=qs[:, :])
                    qrow = stats.tile([1, T], F32, tag="qrow", name="qrow")
                    nc.sync.dma_start(out=qrow[:, :],
                                      in_=srow_v[r0 + 1:r0 + 2, :])
                    mxrow = stats.tile([1, T], F32, tag="mxrow",
                                       name="mxrow")
                    nc.sync.dma_start(out=mxrow[:, :],
                                      in_=srow_v[r0:r0 + 1, :])
                    for half in range(2):
                        bps = psum.tile([128, TH], F32, tag="b", name="bps")
                        nc.tensor.matmul(bps[:, :], ones_r[:, :],
                                         qrow[0:1, ts(half, TH)],
                                         start=True, stop=True)
                        nc.scalar.activation(qs_b[:, ts(half, TH)], bps,
                                             AF.Copy)
                    return mxrow

                def al_row(mxrow, sqrow, wsc, sqrt_dim, al_b):
                    # per-token dequant scale from [1,T] rows -> al_b bcast
                    nc.vector.tensor_scalar(sqrow, sqrow, 1e-38, None,
                                            OP.max)
                    nc.scalar.activation(sqrow, sqrow, AF.Sqrt)
                    nc.vector.tensor_scalar(sqrow, sqrow, 1e-12, None,
                                            OP.max)
                    nc.vector.reciprocal(sqrow, sqrow)
                    nc.vector.tensor_tensor(sqrow, mxrow, sqrow, OP.mult)
                    nc.vector.tensor_scalar(sqrow, sqrow, sqrt_dim, 1e-5,
                                            OP.mult, OP.max)
                    nc.vector.tensor_scalar(sqrow, sqrow, wsc[0:1, 1:2],
                                            1.0 / 127.0, OP.mult, OP.mult)
                    nc.gpsimd.partition_broadcast(al_b[:, :], sqrow[:, :])

                def scale_trigger(wps, ar_in, ar_out):
                    # column partial sums -> scalar -> AllReduce trigger
                    wpad = stats.tile([128, 128], F32, tag="wpad",
                                      name="wpad")
                    nc.vector.memset(wpad, 0.0)
                    nc.vector.reduce_sum(wpad[:, 0:1], wps, axis=AX)
                    trw = psum.tile([128, 128], F32, tag="b", name="trw")
                    nc.tensor.transpose(trw[:, :], wpad[:, :], ident[:, :])
                    wred = stats.tile([8, 1], F32, tag="wred", name="wred")
                    nc.vector.memset(wred, 0.0)
                    nc.vector.reduce_sum(wred[0:1, :], trw[0:1, :], axis=AX)
                    nc.sync.dma_start(out=ar_in.ap()[0:8], in_=wred[:, :])
                    nc.gpsimd.collective_compute(
                        "AllReduce", OP.add,
                        replica_groups=[list(range(n_cores))],
                        ins=[ar_in.ap().opt()], outs=[ar_out.ap().opt()])

                def scale_readback(ar_out, wsc):
                    wrow = stats.tile([1, 1], F32, tag="wrow", name="wrow")
                    nc.sync.dma_start(out=wrow[:, :], in_=ar_out.ap()[0:1])
                    mrow = stats.tile([1, 2], F32, tag="mrow", name="mrow")
                    nc.vector.tensor_scalar(mrow[:, 1:2], wrow, inv_cnt,
                                            1e-5, OP.mult, OP.max)
                    nc.vector.reciprocal(mrow[:, 0:1], mrow[:, 1:2])
                    nc.gpsimd.partition_broadcast(wsc[:, :], mrow[:, :])

                with tc.tile_pool(name="bc", bufs=2) as bc:
                    # -- w1 scale-partial shard reads + AllReduce first:
                    #    the rendezvous overlaps the x-stats pass --
                    wps1 = stats.tile([128, NSH1], F32, tag="wps1",
                                      name="wps1")
                    for a in range(NSH1):
                        wtmp = bc.tile([128, KD * 128], F32, tag="w1f",
                                       bufs=4, name="wtmp")
                        nc.sync.dma_start(out=wtmp[:, :], in_=w1sh_v[a])
                        nc.vector.tensor_reduce(
                            out=wps1[:, a:a + 1], in_=wtmp[:, :], axis=AX,
                            op=OP.add, apply_absolute_value=True)
                    scale_trigger(wps1, ar1_in, ar1_out)

                    # -- x strip DMAs (kept resident in SBUF) --
                    xk_tiles = {}
                    for k in range(KD):
                        xk = bc.tile([128, T], F32, tag="xk", bufs=KD,
                                     name="xk")
                        nc.sync.dma_start(out=xk[:, :], in_=xT_t[k])
                        xk_tiles[k] = xk

                    # -- prefetch first layer-1 weight strips --
                    w1f_tiles = {}

                    def load_w1f(i):
                        t = bc.tile([128, KD * 128], F32, tag="w1f", bufs=4,
                                    name="w1f")
                        nc.sync.dma_start(out=t[:, :], in_=w1p_v[i])
                        w1f_tiles[i] = t

                    for i in range(4):
                        load_w1f(i)

                    # ====== Stage B: x stats ======
                    # absmax: scalar Abs + single vector max chain
                    am1 = stats.tile([128, T], F32, tag="amp", name="am1")
                    for k in range(KD):
                        xabs = bc.tile([128, T], F32, tag="xabs",
                                       name="xabs")
                        nc.scalar.activation(xabs, xk_tiles[k], AF.Abs)
                        if k == 0:
                            nc.vector.tensor_copy(am1, xabs)
                        else:
                            nc.vector.tensor_tensor(am1, xabs, am1, OP.max)
                    # sum-of-squares: PE ones-matmul accumulation -> [1,T]
                    sqpsA = psum.tile([1, TH], F32, tag="b", name="sqpsA")
                    sqpsB = psum.tile([1, TH], F32, tag="b", name="sqpsB")
                    for k in range(KD):
                        xsq = bc.tile([128, T], BF16, tag="xsq", name="xsq")
                        nc.scalar.activation(xsq, xk_tiles[k], AF.Square)
                        nc.tensor.matmul(sqpsA[:, :], ones_c[:, :],
                                         xsq[:, 0:TH], start=(k == 0),
                                         stop=(k == KD - 1))
                        nc.tensor.matmul(sqpsB[:, :], ones_c[:, :],
                                         xsq[:, TH:T], start=(k == 0),
                                         stop=(k == KD - 1))
                    Mx1 = stats.tile([128, TJ], F32, tag="Mx", name="Mx1")
                    part_reduce(am1, Mx1, OP.max)
                    mxrow1 = qs_part(Mx1, 0, qs1_b)
                    scale_readback(ar1_out, wsc1)
                    sqrow1 = stats.tile([1, T], F32, tag="sqrow",
                                        name="sqrow1")
                    nc.scalar.activation(sqrow1[:, 0:TH], sqpsA, AF.Copy)
                    nc.scalar.activation(sqrow1[:, TH:T], sqpsB, AF.Copy)
                    al_row(mxrow1, sqrow1, wsc1, sqrt_d, al1_b)

                    # ====== Stage B2: x quant (in SBUF, no re-read) ======
                    xqT = bc.tile([128, KD, T], BF16, tag="xqT", bufs=1,
                                  name="xqT")
                    for k in range(KD):
                        xk = xk_tiles.pop(k)
                        nc.vector.tensor_tensor(xk, xk, qs1_b, OP.mult)
                        nc.vector.tensor_scalar(xqT[:, k, :], xk, C_ROUND,
                                                C_ROUND, OP.add, OP.subtract)

                    # ====== Stage C: layer 1 (+ w2 scale interleaved) ======
                    am2 = stats.tile([128, T], F32, tag="amp", name="am2")
                    sq2 = stats.tile([128, T], F32, tag="sq", name="sq2")
                    wps2 = stats.tile([128, NCH], F32, tag="wps2",
                                      name="wps2")
                    w1q_tiles = {}

                    def quant_w1(i):
                        # ternary-quantize strip i (consumes w1f_tiles[i])
                        w1f = w1f_tiles.pop(i)
                        w1ff = w1f[:, :]
                        nc.scalar.activation(w1ff, w1ff, AF.Copy,
                                             scale=wsc1[:, 0:1])
                        nc.vector.tensor_scalar(w1ff, w1ff, W_CLIP, -W_CLIP,
                                                OP.min, OP.max)
                        w1q = bc.tile([128, KD, 128], BF16, tag="w1q",
                                      bufs=4, name="w1q")
                        nc.vector.tensor_scalar(
                            w1q.rearrange("p k j -> p (k j)"), w1ff, C_ROUND,
                            C_ROUND, OP.add, OP.subtract)
                        w1q_tiles[i] = w1q

                    quant_w1(0)
                    for i in range(KI):
                        # quantize NEXT strip first: keeps the scalar/vector
                        # queues from blocking strip i+1's weights behind
                        # strip i's gelu/stats (in-order queues)
                        if i + 1 < KI:
                            quant_w1(i + 1)
                        if i + 4 < KI:
                            load_w1f(i + 4)
                        w1q = w1q_tiles.pop(i)
                        hpsA = psum.tile([128, TH], F32, tag="b",
                                         name="hpsA")
                        hpsB = psum.tile([128, TH], F32, tag="b",
                                         name="hpsB")
                        for k in range(KD):
                            nc.tensor.matmul(hpsA[:, :], w1q[:, k, :],
                                             xqT[:, k, 0:TH],
                                             start=(k == 0),
                                             stop=(k == KD - 1))
                        for k in range(KD):
                            nc.tensor.matmul(hpsB[:, :], w1q[:, k, :],
                                             xqT[:, k, TH:T],
                                             start=(k == 0),
                                             stop=(k == KD - 1))
                        nc.vector.tensor_tensor(hpsA, hpsA, al1_b[:, 0:TH],
                                                OP.mult)
                        nc.vector.tensor_tensor(hpsB, hpsB, al1_b[:, TH:T],
                                                OP.mult)
                        h_sb = bc.tile([128, T], F32, tag="h", bufs=3,
                                       name="h_sb")
                        nc.scalar.activation(h_sb[:, 0:TH], hpsA, AF.Gelu)
                        nc.scalar.activation(h_sb[:, TH:T], hpsB, AF.Gelu)
                        nc.sync.dma_start(out=h_w[i], in_=h_sb[:, :])
                        habs = bc.tile([128, T], F32, tag="xabs",
                                       name="habs")
                        nc.scalar.activation(habs, h_sb, AF.Abs)
                        if i == 0:
                            nc.vector.tensor_copy(am2, habs)
                        else:
                            nc.vector.tensor_tensor(am2, habs, am2, OP.max)
                        hsq = bc.tile([128, T], BF16, tag="hsq", name="hsq")
                        nc.scalar.activation(hsq, h_sb, AF.Square)
                        if i == 0:
                            nc.vector.tensor_copy(sq2, hsq)
                        else:
                            nc.vector.tensor_tensor(sq2, hsq, sq2, OP.add)
                        # interleaved w2 scale partial (this core's strip)
                        if i < NCH:
                            wtmp2 = bc.tile([128, CW], F32, tag="w1f",
                                            bufs=4, name="wtmp2")
                            nc.sync.dma_start(
                                out=wtmp2[:, :],
                                in_=w2sh_v[:, CW * i:CW * (i + 1)])
                            nc.vector.tensor_reduce(
                                out=wps2[:, i:i + 1], in_=wtmp2[:, :],
                                axis=AX, op=OP.add,
                                apply_absolute_value=True)
                        elif i == NCH:
                            scale_trigger(wps2, ar2_in, ar2_out)
                        elif i == 56:
                            scale_readback(ar2_out, wsc2)

                    # ---- mid stats finalize ----
                    qs2_b = glob.tile([128, T], F32, tag="qsb",
                                      name="qs2_b")
                    al2_b = glob.tile([128, T], F32, tag="alb",
                                      name="al2_b")
                    Mx2 = stats.tile([128, TJ], F32, tag="Mx", name="Mx2")
                    part_reduce(am2, Mx2, OP.max)
                    Sq2 = stats.tile([128, TJ], F32, tag="Sq", name="Sq2")
                    part_reduce(sq2, Sq2, OP.add)
                    mxrow2 = qs_part(Mx2, 3, qs2_b)
                    nc.sync.dma_start(out=stok_v[5], in_=Sq2[:, :])
                    sqrow2 = stats.tile([1, T], F32, tag="sqrow",
                                        name="sqrow2")
                    nc.sync.dma_start(out=sqrow2[:, :], in_=srow_v[5:6, :])
                    al_row(mxrow2, sqrow2, wsc2, sqrt_i, al2_b)

            # ============ Stage D: quantize h, layer 2 ============
            with tc.tile_pool(name="l2", bufs=2) as l2:
                w2qc_tiles = {}

                def issue_w2_chunk(m, q):
                    w2f = l2.tile([128, CW], F32, tag="w2f", bufs=2,
                                  name="w2f")
                    nc.sync.dma_start(
                        out=w2f[:, :],
                        in_=w2p_v[m][:, CW * q:CW * (q + 1)])
                    nc.scalar.activation(w2f[:, :], w2f[:, :], AF.Copy,
                                         scale=wsc2[:, 0:1])
                    nc.vector.tensor_scalar(w2f, w2f, W_CLIP, -W_CLIP,
                                            OP.min, OP.max)
                    t = l2.tile([128, KPC, 256], BF16, tag="w2q", bufs=8,
                                name="w2q")
                    nc.vector.tensor_scalar(
                        t.rearrange("p a c -> p (a c)"), w2f, C_ROUND,
                        C_ROUND, OP.add, OP.subtract)
                    w2qc_tiles[(m, q)] = t

                hqT = l2.tile([128, KI, T], BF16, tag="hqT", bufs=1,
                              name="hqT")
                for k2 in range(KI):
                    hk = l2.tile([128, T], F32, tag="hrd", bufs=4,
                                 name="hk")
                    nc.sync.dma_start(out=hk[:, :], in_=h_w[k2])
                    nc.vector.tensor_tensor(hk, hk, qs2_b, OP.mult)
                    nc.vector.tensor_scalar(hqT[:, k2, :], hk, C_ROUND,
                                            C_ROUND, OP.add, OP.subtract)
                    if k2 % KPC == KPC - 1:
                        issue_w2_chunk(0, k2 // KPC)
                        issue_w2_chunk(1, k2 // KPC)

                def group_out(m, pb):
                    for qq in range(4):
                        jcol = qq >> 1
                        half = qq & 1
                        ob = l2.tile([128, TH], F32, tag="ob", bufs=2,
                                     name="ob")
                        nc.vector.tensor_tensor(
                            ob, pb[qq], al2_b[:, ts(half, TH)], OP.mult)
                        nc.sync.dma_start(
                            out=out_w[2 * m + jcol][:, ts(half, TH)],
                            in_=ob[:, :])

                # groups m=0 and m=1: interleaved k-accumulation so the PE
                # consumes hqT strips at production rate
                for q in range(NCH):
                    issue_w2_chunk(2, q)
                pbA = [psum.tile([128, TH], F32, tag="b",
                                 name=f"pbA{qq}") for qq in range(4)]
                pbB = [psum.tile([128, TH], F32, tag="b",
                                 name=f"pbB{qq}") for qq in range(4)]
                for kk in range(KI):
                    first = (kk == 0)
                    last = (kk == KI - 1)
                    r = kk % KPC
                    for pb, ch in ((pbA, w2qc_tiles[(0, kk // KPC)]),
                                   (pbB, w2qc_tiles[(1, kk // KPC)])):
                        nc.tensor.matmul(pb[0][:, :], ch[:, r, 0:128],
                                         hqT[:, kk, 0:TH],
                                         start=first, stop=last)
                        nc.tensor.matmul(pb[1][:, :], ch[:, r, 0:128],
                                         hqT[:, kk, TH:T],
                                         start=first, stop=last)
                        nc.tensor.matmul(pb[2][:, :], ch[:, r, 128:256],
                                         hqT[:, kk, 0:TH],
                                         start=first, stop=last)
                        nc.tensor.matmul(pb[3][:, :], ch[:, r, 128:256],
                                         hqT[:, kk, TH:T],
                                         start=first, stop=last)
                for q in range(NCH):
                    w2qc_tiles.pop((0, q))
                    w2qc_tiles.pop((1, q))
                group_out(0, pbA)
                group_out(1, pbB)

                for m in range(2, NM):
                    # prefetch NEXT group's chunks before this group's
                    # matmuls/output ops (in-order vector queue)
                    if m + 1 < NM:
                        for q in range(NCH):
                            issue_w2_chunk(m + 1, q)
                    pb = [psum.tile([128, TH], F32, tag="b",
                                    name=f"pb{qq}") for qq in range(4)]
                    for kk in range(KI):
                        ch = w2qc_tiles.pop((m, kk // KPC)) \
                            if kk % KPC == KPC - 1 else \
                            w2qc_tiles[(m, kk // KPC)]
                        r = kk % KPC
                        first = (kk == 0)
                        last = (kk == KI - 1)
                        nc.tensor.matmul(pb[0][:, :], ch[:, r, 0:128],
                                         hqT[:, kk, 0:TH],
                                         start=first, stop=last)
                        nc.tensor.matmul(pb[1][:, :], ch[:, r, 0:128],
                                         hqT[:, kk, TH:T],
                                         start=first, stop=last)
                        nc.tensor.matmul(pb[2][:, :], ch[:, r, 128:256],
                                         hqT[:, kk, 0:TH],
                                         start=first, stop=last)
                        nc.tensor.matmul(pb[3][:, :], ch[:, r, 128:256],
                                         hqT[:, kk, TH:T],
                                         start=first, stop=last)
                    group_out(m, pb)

    nc.compile()  # Bacc passes: EVSEM multi-wait lowering, library loads,
    return nc     # extended-ISA codegen, nop fusion, register alloc


_NC_CACHE = {}


def _get_nc(D, I, T, n_cores):
    key = (D, I, T, n_cores)
    if key not in _NC_CACHE:
        _NC_CACHE[key] = build_bitnet(D, I, T, n_cores)
    return _NC_CACHE[key]


def make_in_maps(x, w1, w2, n_cores=N_CORES):
    """Host-side sharding/layout only (transpose + slicing, no arithmetic)."""
    xf = np.ascontiguousarray(np.asarray(x, dtype=np.float32)).reshape(
        -1, x.shape[-1])
    D = xf.shape[1]
    I = w1.shape[0]
    T = xf.shape[0] // n_cores
    KD = D // 128
    KI = I // 128
    NM = D // 256
    w1np = np.asarray(w1, dtype=np.float32)
    w2np = np.asarray(w2, dtype=np.float32)
    # strip-major stationary layouts (pure element permutations)
    # w1p[s, p, k*128+j] = w1[128s+j, 128k+p]
    w1p = np.ascontiguousarray(
        w1np.reshape(KI, 128, KD, 128).transpose(0, 3, 2, 1)).reshape(
        KI, 128, KD * 128)
    # w2p[m, p, k*256+c] = w2[256m+c, 128k+p]
    w2p = np.ascontiguousarray(
        w2np.reshape(NM, 256, KI, 128).transpose(0, 3, 2, 1)).reshape(
        NM, 128, KI * 256)
    in_maps = []
    for c in range(n_cores):
        xTc = np.ascontiguousarray(xf[c * T:(c + 1) * T].T)  # [D, T]
        in_maps.append({
            "xT": xTc,
            "w1p": w1p,
            "w2p": w2p,
            "w1sh": w1p[c * (KI // n_cores):(c + 1) * (KI // n_cores)],
            "w2sh": w2p[c],
        })
    return in_maps, (D, I, T)


def run_spmd(x, w1, w2, trace=False, **kwargs):
    from concourse.bass_utils import run_bass_kernel_spmd

    B, S, D = x.shape
    in_maps, (D, I, T) = make_in_maps(x, w1, w2, N_CORES)
    nc = _get_nc(D, I, T, N_CORES)
    res = run_bass_kernel_spmd(nc, in_maps, core_ids=list(range(N_CORES)),
                               trace=trace, **kwargs)
    outs = [res.results[c]["outT"].T for c in range(N_CORES)]  # each [T, D]
    out = np.concatenate(outs, axis=0).reshape(B, S, D)
    return np.ascontiguousarray(out, dtype=np.float32), res


def kernel(x, w1, w2):
    out, _ = run_spmd(x, w1, w2, trace=False)
    return out
